# revision 4
# baseline (speedup 1.0000x reference)
"""Position-attention kernel for Trainium2 (8 NeuronCores, SPMD data-parallel).

Math (per batch b):
    q = X Wq ; k = X Wk ; v = X Wv          (X = x[b] reshaped [N, C], N=4096, C=128)
    energy[i, j] = k_i . q_j
    attn = softmax(energy, axis=-1)
    out = gamma * (attn @ v) + X

Kernel restructuring:
    energy = X A X^T  with A = Wq Wk^T   (computed transposed: eT[j, i])
    eT is produced in PSUM with j on partitions, exp'd (shift-invariant) by the
    scalar engine directly into SBUF as bf16 -> already in the right layout to
    be the stationary operand of the attn@v matmul (no transposes anywhere).
    A ones-column appended to v gives the softmax denominator for free.

Sharding: 8 cores = (4 batches) x (2 halves of the 4096 output rows).
"""

import numpy as np

B, Dd, Hh, Ww, C = 4, 16, 16, 16, 128
N = Dd * Hh * Ww            # 4096 sequence positions (j)
NCORES = 8
NI = (B * N) // NCORES      # 2048 output rows per core (i)
NJB = N // 128              # 32 j-blocks
G = 2                       # j-blocks per exp group (PSUM: 2*G + 4 banks = 8)
IC = 512                    # i-chunk (4 accumulator tiles of 128 rows)
NICH = NI // IC             # 4 i-chunks
SHIFT = 32.0                # softmax shift (cancels exactly in normalization)

_NC_CACHE = {}


def _build_nc():
    from contextlib import ExitStack

    import concourse.bacc as bacc
    import concourse.mybir as mybir
    import concourse.tile as tile

    dt = mybir.dt
    nc = bacc.Bacc(target_bir_lowering=False)

    xT_d = nc.declare_dram_parameter("xT", [128, N], dt.float16, isOutput=False)
    xTi_d = nc.declare_dram_parameter("xTi", [128, NI], dt.float16, isOutput=False)
    xres_d = nc.declare_dram_parameter(
        "xres", [NI // 128, 128, 128], dt.float32, isOutput=False
    )
    amat_d = nc.declare_dram_parameter("amat", [128, 128], dt.float16, isOutput=False)
    wv_d = nc.declare_dram_parameter("wv", [128, 128], dt.float16, isOutput=False)
    gam_d = nc.declare_dram_parameter("gam", [1, 1], dt.float32, isOutput=False)
    out_d = nc.declare_dram_parameter(
        "out", [NI // 128, 128, 128], dt.float32, isOutput=True
    )

    with tile.TileContext(nc) as tc, ExitStack() as ctx:
        persist = ctx.enter_context(tc.tile_pool(name="persist", bufs=1))
        xT = persist.tile([128, N], dt.float16)
        xTi = persist.tile([128, NI], dt.float16)
        yt = persist.tile([128, N], dt.float16)
        # v with a ones-column at index 128 (stride padded to 132 elements)
        v = persist.tile([128, NJB, 132], dt.bfloat16)
        amat = persist.tile([128, 128], dt.float16)
        wv = persist.tile([128, 128], dt.float16)
        gam = persist.tile([128, 1], dt.float32)
        shiftb = persist.tile([128, 1], dt.float32)
        nc.vector.memset(shiftb[:], -SHIFT)

        nc.sync.dma_start(out=xT[:], in_=xT_d[:, :])
        nc.sync.dma_start(out=xTi[:], in_=xTi_d[:, :])
        nc.sync.dma_start(out=amat[:], in_=amat_d[:, :])
        nc.sync.dma_start(out=wv[:], in_=wv_d[:, :])
        import concourse.bass as bass

        gam_bcast = bass.AP(
            tensor=gam_d, offset=0, ap=[[0, 128], [1, 1]]
        )
        nc.sync.dma_start(out=gam[:], in_=gam_bcast)
        nc.vector.memset(v[:, :, 128:129], 1.0)

        # ---- prep phase: yt = A^T X^T  and  v = X Wv (both over all j) ----
        with tc.tile_pool(name="prep_psum", bufs=2, space="PSUM") as pp:
            for jc in range(N // 512):
                t = pp.tile([128, 512], dt.float32, tag="yt")
                nc.tensor.matmul(
                    t[:],
                    amat[:],
                    xT[:, jc * 512 : (jc + 1) * 512],
                    start=True,
                    stop=True,
                )
                nc.vector.tensor_copy(out=yt[:, jc * 512 : (jc + 1) * 512], in_=t[:])
            for jq in range(NJB // 4):
                t = pp.tile([128, 4, 128], dt.float32, tag="v")
                for k in range(4):
                    jt = jq * 4 + k
                    nc.tensor.matmul(
                        t[:, k, :],
                        xT[:, jt * 128 : (jt + 1) * 128],
                        wv[:],
                        start=True,
                        stop=True,
                    )
                nc.vector.tensor_copy(out=v[:, jq * 4 : (jq + 1) * 4, 0:128], in_=t[:])

        # ---- main loop ----
        epool = ctx.enter_context(tc.tile_pool(name="epsum", bufs=2, space="PSUM"))
        opool = ctx.enter_context(tc.tile_pool(name="opsum", bufs=1, space="PSUM"))
        ptpool = ctx.enter_context(tc.tile_pool(name="ptp", bufs=3))
        spool = ctx.enter_context(tc.tile_pool(name="small", bufs=8))
        xrpool = ctx.enter_context(tc.tile_pool(name="xrp", bufs=3))
        outpool = ctx.enter_context(tc.tile_pool(name="outp", bufs=3))

        ngroups = (NJB + G - 1) // G
        for icn in range(NICH):
            oa = [
                opool.tile([128, 129], dt.float32, tag=f"oa{k}", name=f"oa{k}_{icn}")
                for k in range(4)
            ]
            for jg in range(ngroups):
                gsz = min(G, NJB - jg * G)
                et = epool.tile([128, G, 512], dt.float32, tag="et")
                for g in range(gsz):
                    jb = jg * G + g
                    nc.tensor.matmul(
                        et[:, g, :],
                        yt[:, jb * 128 : (jb + 1) * 128],
                        xTi[:, icn * IC : (icn + 1) * IC],
                        start=True,
                        stop=True,
                    )
                pt = ptpool.tile([128, G, 512], dt.bfloat16, tag="pt")
                nc.scalar.activation(
                    out=pt[:, :gsz, :],
                    in_=et[:, :gsz, :],
                    func=mybir.ActivationFunctionType.Exp,
                    bias=shiftb[:],
                )
                for g in range(gsz):
                    jb = jg * G + g
                    for it in range(4):
                        nc.tensor.matmul(
                            oa[it][:],
                            pt[:, g, it * 128 : (it + 1) * 128],
                            v[:, jb, 0:129],
                            start=(jb == 0),
                            stop=(jb == NJB - 1),
                        )
            for it in range(4):
                ti = icn * 4 + it
                o = oa[it]
                rs = spool.tile([128, 1], dt.float32, tag="rs")
                nc.vector.reciprocal(rs[:], o[:, 128:129])
                nc.vector.tensor_scalar(
                    out=rs[:],
                    in0=rs[:],
                    scalar1=gam[:],
                    scalar2=None,
                    op0=mybir.AluOpType.mult,
                )
                xr = xrpool.tile([128, 128], dt.float32, tag="xr")
                nc.sync.dma_start(out=xr[:], in_=xres_d[ti])
                ot = outpool.tile([128, 128], dt.float32, tag="ot")
                nc.vector.tensor_scalar(
                    out=ot[:],
                    in0=o[:, 0:128],
                    scalar1=rs[:],
                    scalar2=None,
                    op0=mybir.AluOpType.mult,
                )
                nc.vector.tensor_tensor(
                    out=ot[:], in0=ot[:], in1=xr[:], op=mybir.AluOpType.add
                )
                nc.sync.dma_start(out=out_d[ti], in_=ot[:])

    nc.finalize()
    return nc


def get_nc():
    if "nc" not in _NC_CACHE:
        _NC_CACHE["nc"] = _build_nc()
    return _NC_CACHE["nc"]


def make_in_maps(x, Wq, Wk, Wv, gamma):
    x = np.asarray(x, dtype=np.float32)
    Wq = np.asarray(Wq, dtype=np.float32)
    Wk = np.asarray(Wk, dtype=np.float32)
    Wv = np.asarray(Wv, dtype=np.float32)
    gamma = np.asarray(gamma, dtype=np.float32)

    xf = x.reshape(B, N, C)
    A = (Wq @ Wk.T).astype(np.float16)
    wv16 = Wv.astype(np.float16)
    gam = gamma.reshape(1, 1)

    in_maps = []
    for c in range(NCORES):
        b, ih = c // 2, c % 2
        xT = np.ascontiguousarray(xf[b].T).astype(np.float16)  # [128, 4096]
        sl = slice(ih * NI, (ih + 1) * NI)
        in_maps.append(
            {
                "xT": xT,
                "xTi": np.ascontiguousarray(xT[:, sl]),
                "xres": np.ascontiguousarray(
                    xf[b][sl].reshape(NI // 128, 128, 128)
                ),
                "amat": A,
                "wv": wv16,
                "gam": gam,
            }
        )
    return in_maps


def assemble_out(results):
    outs = [np.asarray(results[c]["out"]).reshape(NI, C) for c in range(NCORES)]
    full = np.stack(
        [np.concatenate([outs[2 * b], outs[2 * b + 1]], axis=0) for b in range(B)]
    )
    return full.reshape(B, Dd, Hh, Ww, C).astype(np.float32)


def kernel(x, Wq, Wk, Wv, gamma):
    from concourse.bass_utils import run_bass_kernel_spmd

    nc = get_nc()
    in_maps = make_in_maps(x, Wq, Wk, Wv, gamma)
    res = run_bass_kernel_spmd(nc, in_maps, core_ids=list(range(NCORES)))
    return assemble_out(res.results)


# revision 5
# speedup vs baseline: 1.0188x; 1.0188x over previous
"""Position-attention kernel for Trainium2 (8 NeuronCores, SPMD data-parallel).

Math (per batch b):
    q = X Wq ; k = X Wk ; v = X Wv          (X = x[b] reshaped [N, C], N=4096, C=128)
    energy[i, j] = k_i . q_j
    attn = softmax(energy, axis=-1)
    out = gamma * (attn @ v) + X

Kernel restructuring:
    energy = X A X^T  with A = Wq Wk^T   (computed transposed: eT[j, i])
    eT is produced in PSUM with j on partitions, exp'd (shift-invariant) by the
    scalar engine directly into SBUF as bf16 -> already in the right layout to
    be the stationary operand of the attn@v matmul (no transposes anywhere).
    A ones-column appended to v gives the softmax denominator for free.

Sharding: 8 cores = (4 batches) x (2 halves of the 4096 output rows).
"""

import numpy as np

B, Dd, Hh, Ww, C = 4, 16, 16, 16, 128
N = Dd * Hh * Ww            # 4096 sequence positions (j)
NCORES = 8
NI = (B * N) // NCORES      # 2048 output rows per core (i)
NJB = N // 128              # 32 j-blocks
G = 6                       # j-blocks per exp group (PSUM: 2*3 + 2 banks = 8)
IC = 256                    # i-chunk (2 accumulator tiles of 128 rows)
NICH = NI // IC             # 8 i-chunks
NIT = IC // 128             # 2 i-tiles per chunk
SHIFT = 32.0                # softmax shift (cancels exactly in normalization)

_NC_CACHE = {}


def _build_nc():
    from contextlib import ExitStack

    import concourse.bacc as bacc
    import concourse.bass as bass
    import concourse.mybir as mybir
    import concourse.tile as tile

    dt = mybir.dt
    nc = bacc.Bacc(target_bir_lowering=False)

    xT_d = nc.declare_dram_parameter("xT", [128, N], dt.float16, isOutput=False)
    xTi_d = nc.declare_dram_parameter("xTi", [128, NI], dt.float16, isOutput=False)
    xres_d = nc.declare_dram_parameter(
        "xres", [NI // 128, 128, 128], dt.float32, isOutput=False
    )
    amat_d = nc.declare_dram_parameter("amat", [128, 128], dt.float16, isOutput=False)
    wv_d = nc.declare_dram_parameter("wv", [128, 128], dt.float16, isOutput=False)
    gam_d = nc.declare_dram_parameter("gam", [1, 1], dt.float32, isOutput=False)
    out_d = nc.declare_dram_parameter(
        "out", [NI // 128, 128, 128], dt.float32, isOutput=True
    )

    NCH = N // 512  # 8 column chunks for xT / yt / v
    with tile.TileContext(nc) as tc, ExitStack() as ctx:
        persist = ctx.enter_context(tc.tile_pool(name="persist", bufs=1))

        # warm up the exp table load while DMAs run
        dummy = persist.tile([1, 1], dt.float32)
        nc.vector.memset(dummy[:], 0.0)
        nc.scalar.activation(
            out=dummy[:], in_=dummy[:], func=mybir.ActivationFunctionType.Exp
        )

        xt_ch = []
        yt_ch = []
        v_ch = []
        for jc in range(NCH):
            t = persist.tile([128, 512], dt.float16, name=f"xt{jc}")
            nc.sync.dma_start(out=t[:], in_=xT_d[:, jc * 512 : (jc + 1) * 512])
            xt_ch.append(t)
            yt_ch.append(persist.tile([128, 512], dt.float16, name=f"yt{jc}"))
            v_ch.append(persist.tile([128, 4, 132], dt.bfloat16, name=f"v{jc}"))
        xTi = persist.tile([128, NI], dt.float16)
        nc.sync.dma_start(out=xTi[:], in_=xTi_d[:, :])
        amat = persist.tile([128, 128], dt.float16)
        nc.sync.dma_start(out=amat[:], in_=amat_d[:, :])
        wv = persist.tile([128, 128], dt.float16)
        nc.sync.dma_start(out=wv[:], in_=wv_d[:, :])
        gam = persist.tile([128, 1], dt.float32)
        gam_ap = gam_d[:, :]
        nc.sync.dma_start(
            out=gam[:],
            in_=bass.AP(tensor=gam_ap.tensor, offset=gam_ap.offset, ap=[[0, 128], [1, 1]]),
        )
        shiftb = persist.tile([128, 1], dt.float32)
        nc.vector.memset(shiftb[:], -SHIFT)
        for jc in range(NCH):
            nc.vector.memset(v_ch[jc][:, :, 128:129], 1.0)

        # ---- prep phase: yt = A^T X^T  and  v = X Wv (both over all j) ----
        with tc.tile_pool(name="prep_psum", bufs=2, space="PSUM") as pp:
            for jc in range(NCH):
                t = pp.tile([128, 512], dt.float32, tag="yt", name=f"ytp{jc}")
                nc.tensor.matmul(t[:], amat[:], xt_ch[jc][:], start=True, stop=True)
                nc.vector.tensor_copy(out=yt_ch[jc][:], in_=t[:])
            for jc in range(NCH):
                t = pp.tile([128, 4, 128], dt.float32, tag="v", name=f"vp{jc}")
                for k in range(4):
                    nc.tensor.matmul(
                        t[:, k, :],
                        xt_ch[jc][:, k * 128 : (k + 1) * 128],
                        wv[:],
                        start=True,
                        stop=True,
                    )
                nc.vector.tensor_copy(out=v_ch[jc][:, :, 0:128], in_=t[:])

        # ---- main loop ----
        epool = ctx.enter_context(tc.tile_pool(name="epsum", bufs=2, space="PSUM"))
        opool = ctx.enter_context(tc.tile_pool(name="opsum", bufs=1, space="PSUM"))
        ptpool = ctx.enter_context(tc.tile_pool(name="ptp", bufs=3))
        spool = ctx.enter_context(tc.tile_pool(name="small", bufs=8))
        xrpool = ctx.enter_context(tc.tile_pool(name="xrp", bufs=3))
        outpool = ctx.enter_context(tc.tile_pool(name="outp", bufs=3))

        ngroups = (NJB + G - 1) // G
        for icn in range(NICH):
            oa = [
                opool.tile([128, 129], dt.float32, tag=f"oa{k}", name=f"oa{k}_{icn}")
                for k in range(NIT)
            ]
            for jg in range(ngroups):
                gsz = min(G, NJB - jg * G)
                et = epool.tile([128, G, IC], dt.float32, tag="et", name=f"et{icn}_{jg}")
                for g in range(gsz):
                    jb = jg * G + g
                    nc.tensor.matmul(
                        et[:, g, :],
                        yt_ch[jb // 4][:, (jb % 4) * 128 : (jb % 4 + 1) * 128],
                        xTi[:, icn * IC : (icn + 1) * IC],
                        start=True,
                        stop=True,
                    )
                pt = ptpool.tile([128, G, IC], dt.bfloat16, tag="pt", name=f"pt{icn}_{jg}")
                nc.scalar.activation(
                    out=pt[:, :gsz, :],
                    in_=et[:, :gsz, :],
                    func=mybir.ActivationFunctionType.Exp,
                    bias=shiftb[:],
                )
                for g in range(gsz):
                    jb = jg * G + g
                    for it in range(NIT):
                        nc.tensor.matmul(
                            oa[it][:],
                            pt[:, g, it * 128 : (it + 1) * 128],
                            v_ch[jb // 4][:, jb % 4, 0:129],
                            start=(jb == 0),
                            stop=(jb == NJB - 1),
                        )
            for it in range(NIT):
                ti = icn * NIT + it
                o = oa[it]
                rs = spool.tile([128, 1], dt.float32, tag="rs", name=f"rs{ti}")
                nc.vector.reciprocal(rs[:], o[:, 128:129])
                nc.vector.tensor_scalar(
                    out=rs[:],
                    in0=rs[:],
                    scalar1=gam[:],
                    scalar2=None,
                    op0=mybir.AluOpType.mult,
                )
                xr = xrpool.tile([128, 128], dt.float32, tag="xr", name=f"xr{ti}")
                nc.sync.dma_start(out=xr[:], in_=xres_d[ti])
                ot = outpool.tile([128, 128], dt.float32, tag="ot", name=f"ot{ti}")
                nc.vector.tensor_scalar(
                    out=ot[:],
                    in0=o[:, 0:128],
                    scalar1=rs[:],
                    scalar2=None,
                    op0=mybir.AluOpType.mult,
                )
                nc.vector.tensor_tensor(
                    out=ot[:], in0=ot[:], in1=xr[:], op=mybir.AluOpType.add
                )
                nc.sync.dma_start(out=out_d[ti], in_=ot[:])

    nc.finalize()
    return nc


def get_nc():
    if "nc" not in _NC_CACHE:
        _NC_CACHE["nc"] = _build_nc()
    return _NC_CACHE["nc"]


def make_in_maps(x, Wq, Wk, Wv, gamma):
    x = np.asarray(x, dtype=np.float32)
    Wq = np.asarray(Wq, dtype=np.float32)
    Wk = np.asarray(Wk, dtype=np.float32)
    Wv = np.asarray(Wv, dtype=np.float32)
    gamma = np.asarray(gamma, dtype=np.float32)

    xf = x.reshape(B, N, C)
    A = (Wq @ Wk.T).astype(np.float16)
    wv16 = Wv.astype(np.float16)
    gam = gamma.reshape(1, 1)

    in_maps = []
    for c in range(NCORES):
        b, ih = c // 2, c % 2
        xT = np.ascontiguousarray(xf[b].T).astype(np.float16)  # [128, 4096]
        sl = slice(ih * NI, (ih + 1) * NI)
        in_maps.append(
            {
                "xT": xT,
                "xTi": np.ascontiguousarray(xT[:, sl]),
                "xres": np.ascontiguousarray(
                    xf[b][sl].reshape(NI // 128, 128, 128)
                ),
                "amat": A,
                "wv": wv16,
                "gam": gam,
            }
        )
    return in_maps


def assemble_out(results):
    outs = [np.asarray(results[c]["out"]).reshape(NI, C) for c in range(NCORES)]
    full = np.stack(
        [np.concatenate([outs[2 * b], outs[2 * b + 1]], axis=0) for b in range(B)]
    )
    return full.reshape(B, Dd, Hh, Ww, C).astype(np.float32)


def kernel(x, Wq, Wk, Wv, gamma):
    from concourse.bass_utils import run_bass_kernel_spmd

    nc = get_nc()
    in_maps = make_in_maps(x, Wq, Wk, Wv, gamma)
    res = run_bass_kernel_spmd(nc, in_maps, core_ids=list(range(NCORES)))
    return assemble_out(res.results)


# revision 6
# speedup vs baseline: 1.0403x; 1.0212x over previous
"""Position-attention kernel for Trainium2 (8 NeuronCores, SPMD data-parallel).

Math (per batch b):
    q = X Wq ; k = X Wk ; v = X Wv          (X = x[b] reshaped [N, C], N=4096, C=128)
    energy[i, j] = k_i . q_j
    attn = softmax(energy, axis=-1)
    out = gamma * (attn @ v) + X

Kernel restructuring:
    energy = X A X^T  with A = Wq Wk^T   (computed transposed: eT[j, i])
    eT is produced in PSUM with j on partitions, exp'd (shift-invariant) by the
    scalar engine directly into SBUF as bf16 -> already in the right layout to
    be the stationary operand of the attn@v matmul (no transposes anywhere).
    A ones-column appended to v gives the softmax denominator for free.

Sharding: 8 cores = (4 batches) x (2 halves of the 4096 output rows).
"""

import numpy as np

B, Dd, Hh, Ww, C = 4, 16, 16, 16, 128
N = Dd * Hh * Ww            # 4096 sequence positions (j)
NCORES = 8
NI = (B * N) // NCORES      # 2048 output rows per core (i)
NJB = N // 128              # 32 j-blocks
G = 6                       # j-blocks per exp group (PSUM: 2*3 + 2 banks = 8)
IC = 256                    # i-chunk (2 accumulator tiles of 128 rows)
NICH = NI // IC             # 8 i-chunks
NIT = IC // 128             # 2 i-tiles per chunk
SHIFT = 32.0                # softmax shift (cancels exactly in normalization)

_NC_CACHE = {}


def _build_nc():
    from contextlib import ExitStack

    import concourse.bacc as bacc
    import concourse.bass as bass
    import concourse.mybir as mybir
    import concourse.tile as tile

    dt = mybir.dt
    nc = bacc.Bacc(target_bir_lowering=False)

    xT_d = nc.declare_dram_parameter("xT", [128, N], dt.float16, isOutput=False)
    xTi_d = nc.declare_dram_parameter("xTi", [128, NI], dt.float16, isOutput=False)
    xres_d = nc.declare_dram_parameter(
        "xres", [NI // 128, 128, 128], dt.float32, isOutput=False
    )
    # weights packed [A | Wv] along columns to halve DMA count
    aw_d = nc.declare_dram_parameter("aw", [128, 256], dt.float16, isOutput=False)
    gam_d = nc.declare_dram_parameter("gam", [1, 1], dt.float32, isOutput=False)
    out_d = nc.declare_dram_parameter(
        "out", [NI // 128, 128, 128], dt.float32, isOutput=True
    )

    NCH = N // 512  # 8 column chunks for xT / yt / v
    with tile.TileContext(nc) as tc, ExitStack() as ctx:
        persist = ctx.enter_context(tc.tile_pool(name="persist", bufs=1))

        # warm up the exp table load while DMAs run
        dummy = persist.tile([1, 1], dt.float32)
        nc.vector.memset(dummy[:], 0.0)
        nc.scalar.activation(
            out=dummy[:], in_=dummy[:], func=mybir.ActivationFunctionType.Exp
        )
        # zeroed operand for PE-warmup matmuls
        warm = persist.tile([128, 128], dt.float16)
        nc.vector.memset(warm[:], 0.0)

        aw = persist.tile([128, 2, 128], dt.float16)
        nc.sync.dma_start(out=aw[:], in_=aw_d[:, :])
        xt_ch = []
        yt_ch = []
        v_ch = []
        for jc in range(NCH):
            t = persist.tile([128, 512], dt.float16, name=f"xt{jc}")
            eng = nc.sync if jc < 4 else nc.scalar
            eng.dma_start(out=t[:], in_=xT_d[:, jc * 512 : (jc + 1) * 512])
            xt_ch.append(t)
            yt_ch.append(persist.tile([128, 512], dt.float16, name=f"yt{jc}"))
            v_ch.append(persist.tile([128, 4, 132], dt.bfloat16, name=f"v{jc}"))
        xTi = persist.tile([128, NI], dt.float16)
        nc.scalar.dma_start(out=xTi[:], in_=xTi_d[:, :])
        gam = persist.tile([128, 1], dt.float32)
        gam_ap = gam_d[:, :]
        nc.gpsimd.dma_start(
            out=gam[:],
            in_=bass.AP(
                tensor=gam_ap.tensor, offset=gam_ap.offset, ap=[[0, 128], [1, 1]]
            ),
        )
        shiftb = persist.tile([128, 1], dt.float32)
        nc.vector.memset(shiftb[:], -SHIFT)
        for jc in range(NCH):
            nc.vector.memset(v_ch[jc][:, :, 128:129], 1.0)

        amat = aw[:, 0, :]
        wv = aw[:, 1, :]

        # ---- prep phase: yt = A^T X^T  and  v = X Wv (both over all j) ----
        with tc.tile_pool(name="prep_psum", bufs=2, space="PSUM") as pp:
            # dummy matmuls to trip the PE HAM clock gate while DMAs land
            wt = pp.tile([128, 512], dt.float32, tag="yt", name="warmp")
            for r in range(8):
                nc.tensor.matmul(
                    wt[:, r * 64 : (r + 1) * 64],
                    warm[:],
                    warm[:, 0:64],
                    start=True,
                    stop=True,
                )
            for jc in range(NCH):
                t = pp.tile([128, 512], dt.float32, tag="yt", name=f"ytp{jc}")
                nc.tensor.matmul(t[:], amat, xt_ch[jc][:], start=True, stop=True)
                nc.vector.tensor_copy(out=yt_ch[jc][:], in_=t[:])
            for jc in range(NCH):
                t = pp.tile([128, 4, 128], dt.float32, tag="v", name=f"vp{jc}")
                for k in range(4):
                    nc.tensor.matmul(
                        t[:, k, :],
                        xt_ch[jc][:, k * 128 : (k + 1) * 128],
                        wv,
                        start=True,
                        stop=True,
                    )
                nc.vector.tensor_copy(out=v_ch[jc][:, :, 0:128], in_=t[:])

        # ---- main loop ----
        epool = ctx.enter_context(tc.tile_pool(name="epsum", bufs=2, space="PSUM"))
        opool = ctx.enter_context(tc.tile_pool(name="opsum", bufs=1, space="PSUM"))
        ptpool = ctx.enter_context(tc.tile_pool(name="ptp", bufs=3))
        spool = ctx.enter_context(tc.tile_pool(name="small", bufs=8))
        osb_pool = ctx.enter_context(tc.tile_pool(name="osb", bufs=3))
        xrpool = ctx.enter_context(tc.tile_pool(name="xrp", bufs=3))
        outpool = ctx.enter_context(tc.tile_pool(name="outp", bufs=3))

        ngroups = (NJB + G - 1) // G

        def emit_energy(icn, jg, gsz):
            et = epool.tile(
                [128, G, IC], dt.float32, tag="et", name=f"et{icn}_{jg}"
            )
            for g in range(gsz):
                jb = jg * G + g
                nc.tensor.matmul(
                    et[:, g, :],
                    yt_ch[jb // 4][:, (jb % 4) * 128 : (jb % 4 + 1) * 128],
                    xTi[:, icn * IC : (icn + 1) * IC],
                    start=True,
                    stop=True,
                )
            return et

        for icn in range(NICH):
            oa = [
                opool.tile([128, 129], dt.float32, tag=f"oa{k}", name=f"oa{k}_{icn}")
                for k in range(NIT)
            ]
            ets = {0: emit_energy(icn, 0, min(G, NJB))}
            for jg in range(ngroups):
                gsz = min(G, NJB - jg * G)
                if jg + 1 < ngroups:
                    # keep PE one group ahead of the exp -> attn@v chain
                    ets[jg + 1] = emit_energy(icn, jg + 1, min(G, NJB - (jg + 1) * G))
                et = ets.pop(jg)
                pt = ptpool.tile(
                    [128, G, IC], dt.bfloat16, tag="pt", name=f"pt{icn}_{jg}"
                )
                nc.scalar.activation(
                    out=pt[:, :gsz, :],
                    in_=et[:, :gsz, :],
                    func=mybir.ActivationFunctionType.Exp,
                    bias=shiftb[:],
                )
                for g in range(gsz):
                    jb = jg * G + g
                    for it in range(NIT):
                        nc.tensor.matmul(
                            oa[it][:],
                            pt[:, g, it * 128 : (it + 1) * 128],
                            v_ch[jb // 4][:, jb % 4, 0:129],
                            start=(jb == 0),
                            stop=(jb == NJB - 1),
                        )
            for it in range(NIT):
                ti = icn * NIT + it
                # single fast PSUM read frees the accumulator bank quickly
                osb = osb_pool.tile([128, 129], dt.float32, tag="osb", name=f"osb{ti}")
                nc.vector.tensor_copy(out=osb[:], in_=oa[it][:])
                rs = spool.tile([128, 1], dt.float32, tag="rs", name=f"rs{ti}")
                nc.vector.reciprocal(rs[:], osb[:, 128:129])
                nc.vector.tensor_scalar(
                    out=rs[:],
                    in0=rs[:],
                    scalar1=gam[:],
                    scalar2=None,
                    op0=mybir.AluOpType.mult,
                )
                xr = xrpool.tile([128, 128], dt.float32, tag="xr", name=f"xr{ti}")
                nc.gpsimd.dma_start(out=xr[:], in_=xres_d[ti])
                ot = outpool.tile([128, 128], dt.float32, tag="ot", name=f"ot{ti}")
                nc.vector.tensor_scalar(
                    out=ot[:],
                    in0=osb[:, 0:128],
                    scalar1=rs[:],
                    scalar2=None,
                    op0=mybir.AluOpType.mult,
                )
                nc.vector.tensor_tensor(
                    out=ot[:], in0=ot[:], in1=xr[:], op=mybir.AluOpType.add
                )
                nc.sync.dma_start(out=out_d[ti], in_=ot[:])

    nc.finalize()
    return nc


def get_nc():
    if "nc" not in _NC_CACHE:
        _NC_CACHE["nc"] = _build_nc()
    return _NC_CACHE["nc"]


def make_in_maps(x, Wq, Wk, Wv, gamma):
    x = np.asarray(x, dtype=np.float32)
    Wq = np.asarray(Wq, dtype=np.float32)
    Wk = np.asarray(Wk, dtype=np.float32)
    Wv = np.asarray(Wv, dtype=np.float32)
    gamma = np.asarray(gamma, dtype=np.float32)

    xf = x.reshape(B, N, C)
    A = (Wq @ Wk.T).astype(np.float16)
    aw = np.ascontiguousarray(
        np.concatenate([A, Wv.astype(np.float16)], axis=1)
    )  # [128, 256]
    gam = gamma.reshape(1, 1)

    in_maps = []
    for c in range(NCORES):
        b, ih = c // 2, c % 2
        xT = np.ascontiguousarray(xf[b].T).astype(np.float16)  # [128, 4096]
        sl = slice(ih * NI, (ih + 1) * NI)
        in_maps.append(
            {
                "xT": xT,
                "xTi": np.ascontiguousarray(xT[:, sl]),
                "xres": np.ascontiguousarray(
                    xf[b][sl].reshape(NI // 128, 128, 128)
                ),
                "aw": aw,
                "gam": gam,
            }
        )
    return in_maps


def assemble_out(results):
    outs = [np.asarray(results[c]["out"]).reshape(NI, C) for c in range(NCORES)]
    full = np.stack(
        [np.concatenate([outs[2 * b], outs[2 * b + 1]], axis=0) for b in range(B)]
    )
    return full.reshape(B, Dd, Hh, Ww, C).astype(np.float32)


def kernel(x, Wq, Wk, Wv, gamma):
    from concourse.bass_utils import run_bass_kernel_spmd

    nc = get_nc()
    in_maps = make_in_maps(x, Wq, Wk, Wv, gamma)
    res = run_bass_kernel_spmd(nc, in_maps, core_ids=list(range(NCORES)))
    return assemble_out(res.results)


# revision 7
# speedup vs baseline: 1.0810x; 1.0391x over previous
"""Position-attention kernel for Trainium2 (8 NeuronCores, SPMD data-parallel).

Math (per batch b):
    q = X Wq ; k = X Wk ; v = X Wv          (X = x[b] reshaped [N, C], N=4096, C=128)
    energy[i, j] = k_i . q_j
    attn = softmax(energy, axis=-1)
    out = gamma * (attn @ v) + X

Kernel restructuring:
    energy = X A X^T with A = Wq Wk^T, computed transposed as
    eT[j, i] = sum_c xT[c, j] * w[c, i]  where  w = A @ X_i^T  (tiny prep matmul).
    eT lands in PSUM with j on partitions and is exp'd (softmax is shift/scale
    invariant) by the scalar engine directly into SBUF as bf16 -> already in
    the right layout to be the stationary operand of the attn@v matmul (no
    transposes anywhere). A ones-column appended to v gives the softmax
    denominator for free.

Sharding: 8 cores = (4 batches) x (2 halves of the 4096 output rows).
"""

import numpy as np

B, Dd, Hh, Ww, C = 4, 16, 16, 16, 128
N = Dd * Hh * Ww            # 4096 sequence positions (j)
NCORES = 8
NI = (B * N) // NCORES      # 2048 output rows per core (i)
NJB = N // 128              # 32 j-blocks
G = 6                       # j-blocks per exp group (PSUM: 2*3 + 2 banks = 8)
IC = 256                    # i-chunk (2 accumulator tiles of 128 rows)
NICH = NI // IC             # 8 i-chunks
NIT = IC // 128             # 2 i-tiles per chunk
SHIFT = 32.0                # softmax shift (cancels exactly in normalization)

_NC_CACHE = {}


def _build_nc():
    from contextlib import ExitStack

    import concourse.bacc as bacc
    import concourse.bass as bass
    import concourse.mybir as mybir
    import concourse.tile as tile

    dt = mybir.dt
    nc = bacc.Bacc(target_bir_lowering=False)

    xT_d = nc.declare_dram_parameter("xT", [128, N], dt.float16, isOutput=False)
    xTi_d = nc.declare_dram_parameter("xTi", [128, NI], dt.float16, isOutput=False)
    xres_d = nc.declare_dram_parameter(
        "xres", [NI // 128, 128, 128], dt.float32, isOutput=False
    )
    # weights packed [A^T | Wv] along columns to halve DMA count
    aw_d = nc.declare_dram_parameter("aw", [128, 256], dt.float16, isOutput=False)
    gam_d = nc.declare_dram_parameter("gam", [1, 1], dt.float32, isOutput=False)
    out_d = nc.declare_dram_parameter(
        "out", [NI // 128, 128, 128], dt.float32, isOutput=True
    )

    NCH = N // 512   # 8 column chunks of xT
    NWC = NI // 512  # 4 column chunks of xTi / w
    with tile.TileContext(nc) as tc, ExitStack() as ctx:
        persist = ctx.enter_context(tc.tile_pool(name="persist", bufs=1))

        # warm up the exp table load while DMAs run
        dummy = persist.tile([1, 1], dt.float32)
        nc.vector.memset(dummy[:], 0.0)
        nc.scalar.activation(
            out=dummy[:], in_=dummy[:], func=mybir.ActivationFunctionType.Exp
        )
        # zeroed operand for PE-warmup matmuls
        warm = persist.tile([128, 128], dt.float16)
        nc.vector.memset(warm[:], 0.0)

        # sync HW queue: weights then first xT chunks; scalar HW queue: xTi
        # chunks (feed the critical w matmuls) then remaining xT chunks.
        aw = persist.tile([128, 2, 128], dt.float16)
        nc.sync.dma_start(out=aw[:], in_=aw_d[:, :])
        xti_ch = []
        for k in range(NWC):
            t = persist.tile([128, 512], dt.float16, name=f"xti{k}")
            nc.scalar.dma_start(out=t[:], in_=xTi_d[:, k * 512 : (k + 1) * 512])
            xti_ch.append(t)
        xt_ch = []
        v_ch = []
        for jc in range(NCH):
            t = persist.tile([128, 512], dt.float16, name=f"xt{jc}")
            eng = nc.sync if jc < 4 else nc.scalar
            eng.dma_start(out=t[:], in_=xT_d[:, jc * 512 : (jc + 1) * 512])
            xt_ch.append(t)
            v_ch.append(persist.tile([128, 4, 132], dt.bfloat16, name=f"v{jc}"))
        gam = persist.tile([128, 1], dt.float32)
        gam_ap = gam_d[:, :]
        nc.gpsimd.dma_start(
            out=gam[:],
            in_=bass.AP(
                tensor=gam_ap.tensor, offset=gam_ap.offset, ap=[[0, 128], [1, 1]]
            ),
        )
        shiftb = persist.tile([128, 1], dt.float32)
        nc.vector.memset(shiftb[:], -SHIFT)
        for jc in range(NCH):
            nc.vector.memset(v_ch[jc][:, :, 128:129], 1.0)

        at_s = aw[:, 0, :]
        wv_s = aw[:, 1, :]
        w_ch = [persist.tile([128, 512], dt.float16, name=f"w{k}") for k in range(NWC)]

        # ---- prep: w = A X_i^T (critical path) and v = X Wv ----
        with tc.tile_pool(name="prep_psum", bufs=2, space="PSUM") as pp:
            wt = pp.tile([128, 512], dt.float32, tag="w", name="warmp")
            for r in range(8):
                nc.tensor.matmul(
                    wt[:, r * 64 : (r + 1) * 64],
                    warm[:],
                    warm[:, 0:64],
                    start=True,
                    stop=True,
                )
            for k in range(NWC):
                t = pp.tile([128, 512], dt.float32, tag="w", name=f"wp{k}")
                nc.tensor.matmul(t[:], at_s, xti_ch[k][:], start=True, stop=True)
                if k % 2 == 0:
                    nc.vector.tensor_copy(out=w_ch[k][:], in_=t[:])
                else:
                    nc.scalar.copy(out=w_ch[k][:], in_=t[:])
            for jc in range(NCH):
                t = pp.tile([128, 4, 128], dt.float32, tag="v", name=f"vp{jc}")
                for k in range(4):
                    nc.tensor.matmul(
                        t[:, k, :],
                        xt_ch[jc][:, k * 128 : (k + 1) * 128],
                        wv_s,
                        start=True,
                        stop=True,
                    )
                if jc % 2 == 0:
                    nc.vector.tensor_copy(out=v_ch[jc][:, :, 0:128], in_=t[:])
                else:
                    nc.scalar.copy(out=v_ch[jc][:, :, 0:128], in_=t[:])

        # ---- main loop ----
        epool = ctx.enter_context(tc.tile_pool(name="epsum", bufs=2, space="PSUM"))
        opool = ctx.enter_context(tc.tile_pool(name="opsum", bufs=1, space="PSUM"))
        ptpool = ctx.enter_context(tc.tile_pool(name="ptp", bufs=3))
        spool = ctx.enter_context(tc.tile_pool(name="small", bufs=8))
        osb_pool = ctx.enter_context(tc.tile_pool(name="osb", bufs=3))
        xrpool = ctx.enter_context(tc.tile_pool(name="xrp", bufs=3))
        outpool = ctx.enter_context(tc.tile_pool(name="outp", bufs=3))

        ngroups = (NJB + G - 1) // G

        def emit_energy(icn, jg, gsz):
            et = epool.tile([128, G, IC], dt.float32, tag="et", name=f"et{icn}_{jg}")
            wsl = w_ch[icn // 2][:, (icn % 2) * IC : (icn % 2 + 1) * IC]
            for g in range(gsz):
                jb = jg * G + g
                nc.tensor.matmul(
                    et[:, g, :],
                    xt_ch[jb // 4][:, (jb % 4) * 128 : (jb % 4 + 1) * 128],
                    wsl,
                    start=True,
                    stop=True,
                )
            return et

        for icn in range(NICH):
            oa = [
                opool.tile([128, 129], dt.float32, tag=f"oa{k}", name=f"oa{k}_{icn}")
                for k in range(NIT)
            ]
            ets = {0: emit_energy(icn, 0, min(G, NJB))}
            for jg in range(ngroups):
                gsz = min(G, NJB - jg * G)
                if jg + 1 < ngroups:
                    # keep PE one group ahead of the exp -> attn@v chain
                    ets[jg + 1] = emit_energy(icn, jg + 1, min(G, NJB - (jg + 1) * G))
                et = ets.pop(jg)
                pt = ptpool.tile(
                    [128, G, IC], dt.bfloat16, tag="pt", name=f"pt{icn}_{jg}"
                )
                nc.scalar.activation(
                    out=pt[:, :gsz, :],
                    in_=et[:, :gsz, :],
                    func=mybir.ActivationFunctionType.Exp,
                    bias=shiftb[:],
                )
                for g in range(gsz):
                    jb = jg * G + g
                    for it in range(NIT):
                        nc.tensor.matmul(
                            oa[it][:],
                            pt[:, g, it * 128 : (it + 1) * 128],
                            v_ch[jb // 4][:, jb % 4, 0:129],
                            start=(jb == 0),
                            stop=(jb == NJB - 1),
                        )
            for it in range(NIT):
                ti = icn * NIT + it
                # single fast PSUM read frees the accumulator bank quickly
                osb = osb_pool.tile([128, 129], dt.float32, tag="osb", name=f"osb{ti}")
                nc.vector.tensor_copy(out=osb[:], in_=oa[it][:])
                rs = spool.tile([128, 1], dt.float32, tag="rs", name=f"rs{ti}")
                nc.vector.reciprocal(rs[:], osb[:, 128:129])
                nc.vector.tensor_scalar(
                    out=rs[:],
                    in0=rs[:],
                    scalar1=gam[:],
                    scalar2=None,
                    op0=mybir.AluOpType.mult,
                )
                xr = xrpool.tile([128, 128], dt.float32, tag="xr", name=f"xr{ti}")
                nc.gpsimd.dma_start(out=xr[:], in_=xres_d[ti])
                ot = outpool.tile([128, 128], dt.float32, tag="ot", name=f"ot{ti}")
                nc.vector.tensor_scalar(
                    out=ot[:],
                    in0=osb[:, 0:128],
                    scalar1=rs[:],
                    scalar2=None,
                    op0=mybir.AluOpType.mult,
                )
                nc.vector.tensor_tensor(
                    out=ot[:], in0=ot[:], in1=xr[:], op=mybir.AluOpType.add
                )
                nc.sync.dma_start(out=out_d[ti], in_=ot[:])

    nc.finalize()
    return nc


def get_nc():
    if "nc" not in _NC_CACHE:
        _NC_CACHE["nc"] = _build_nc()
    return _NC_CACHE["nc"]


def make_in_maps(x, Wq, Wk, Wv, gamma):
    x = np.asarray(x, dtype=np.float32)
    Wq = np.asarray(Wq, dtype=np.float32)
    Wk = np.asarray(Wk, dtype=np.float32)
    Wv = np.asarray(Wv, dtype=np.float32)
    gamma = np.asarray(gamma, dtype=np.float32)

    xf = x.reshape(B, N, C)
    A = Wq @ Wk.T
    aw = np.ascontiguousarray(
        np.concatenate([A.T.astype(np.float16), Wv.astype(np.float16)], axis=1)
    )  # [128, 256] = [A^T | Wv]
    gam = gamma.reshape(1, 1)

    in_maps = []
    for c in range(NCORES):
        b, ih = c // 2, c % 2
        xT = np.ascontiguousarray(xf[b].T).astype(np.float16)  # [128, 4096]
        sl = slice(ih * NI, (ih + 1) * NI)
        in_maps.append(
            {
                "xT": xT,
                "xTi": np.ascontiguousarray(xT[:, sl]),
                "xres": np.ascontiguousarray(
                    xf[b][sl].reshape(NI // 128, 128, 128)
                ),
                "aw": aw,
                "gam": gam,
            }
        )
    return in_maps


def assemble_out(results):
    outs = [np.asarray(results[c]["out"]).reshape(NI, C) for c in range(NCORES)]
    full = np.stack(
        [np.concatenate([outs[2 * b], outs[2 * b + 1]], axis=0) for b in range(B)]
    )
    return full.reshape(B, Dd, Hh, Ww, C).astype(np.float32)


def kernel(x, Wq, Wk, Wv, gamma):
    from concourse.bass_utils import run_bass_kernel_spmd

    nc = get_nc()
    in_maps = make_in_maps(x, Wq, Wk, Wv, gamma)
    res = run_bass_kernel_spmd(nc, in_maps, core_ids=list(range(NCORES)))
    return assemble_out(res.results)


# revision 9
# speedup vs baseline: 1.1069x; 1.0239x over previous
"""Position-attention kernel for Trainium2 (8 NeuronCores, SPMD data-parallel).

Math (per batch b):
    q = X Wq ; k = X Wk ; v = X Wv          (X = x[b] reshaped [N, C], N=4096, C=128)
    energy[i, j] = k_i . q_j
    attn = softmax(energy, axis=-1)
    out = gamma * (attn @ v) + X

Kernel restructuring:
    energy = X A X^T with A = Wq Wk^T, computed transposed as
    eT[j, i] = sum_c xT[c, j] * w[c, i]  where  w = A @ X_i^T  (tiny prep matmul).
    eT lands in PSUM with j on partitions and is exp'd (softmax is shift/scale
    invariant) by the scalar engine directly into SBUF as bf16 -> already in
    the right layout to be the stationary operand of the attn@v matmul (no
    transposes anywhere). A ones-column appended to v gives the softmax
    denominator for free.

Sharding: 8 cores = (4 batches) x (2 halves of the 4096 output rows).
"""

import numpy as np

B, Dd, Hh, Ww, C = 4, 16, 16, 16, 128
N = Dd * Hh * Ww            # 4096 sequence positions (j)
NCORES = 8
NI = (B * N) // NCORES      # 2048 output rows per core (i)
NJB = N // 128              # 32 j-blocks
G = 6                       # j-blocks per exp group (PSUM: 2*3 + 2 banks = 8)
IC = 256                    # i-chunk (2 accumulator tiles of 128 rows)
NICH = NI // IC             # 8 i-chunks
NIT = IC // 128             # 2 i-tiles per chunk
SHIFT = 32.0                # softmax shift (cancels exactly in normalization)

_NC_CACHE = {}


def _build_nc():
    from contextlib import ExitStack

    import concourse.bacc as bacc
    import concourse.bass as bass
    import concourse.mybir as mybir
    import concourse.tile as tile

    dt = mybir.dt
    nc = bacc.Bacc(target_bir_lowering=False)

    xT_d = nc.declare_dram_parameter("xT", [128, N], dt.float16, isOutput=False)
    xTi_d = nc.declare_dram_parameter("xTi", [128, NI], dt.float16, isOutput=False)
    xres_d = nc.declare_dram_parameter(
        "xres", [NI // 128, 128, 128], dt.float32, isOutput=False
    )
    # weights packed [A^T | Wv] along columns to halve DMA count
    aw_d = nc.declare_dram_parameter("aw", [128, 256], dt.float16, isOutput=False)
    gam_d = nc.declare_dram_parameter("gam", [1, 1], dt.float32, isOutput=False)
    out_d = nc.declare_dram_parameter(
        "out", [NI // 128, 128, 128], dt.float32, isOutput=True
    )

    NCH = N // 512   # 8 column chunks of xT
    NWC = NI // 512  # 4 column chunks of xTi / w
    with tile.TileContext(nc) as tc, ExitStack() as ctx:
        persist = ctx.enter_context(tc.tile_pool(name="persist", bufs=1))

        # warm up the exp table load while DMAs run
        dummy = persist.tile([1, 1], dt.float32)
        nc.vector.memset(dummy[:], 0.0)
        nc.scalar.activation(
            out=dummy[:], in_=dummy[:], func=mybir.ActivationFunctionType.Exp
        )
        # zeroed operand for PE-warmup matmuls
        warm = persist.tile([128, 128], dt.float16)
        nc.vector.memset(warm[:], 0.0)

        # DMA order mirrors the critical path: aw -> xti0 -> xt0 on the sync
        # queue feed the first energy group; the scalar queue carries the rest.
        aw = persist.tile([128, 2, 128], dt.float16)
        nc.sync.dma_start(out=aw[:], in_=aw_d[:, :])
        xti_ch = [
            persist.tile([128, 512], dt.float16, name=f"xti{k}") for k in range(NWC)
        ]
        nc.sync.dma_start(out=xti_ch[0][:], in_=xTi_d[:, 0:512])
        for k in range(1, NWC):
            nc.scalar.dma_start(
                out=xti_ch[k][:], in_=xTi_d[:, k * 512 : (k + 1) * 512]
            )
        xt_ch = []
        v_ch = []
        for jc in range(NCH):
            t = persist.tile([128, 512], dt.float16, name=f"xt{jc}")
            eng = nc.sync if jc < 4 else nc.scalar
            eng.dma_start(out=t[:], in_=xT_d[:, jc * 512 : (jc + 1) * 512])
            xt_ch.append(t)
            v_ch.append(persist.tile([128, 4, 132], dt.bfloat16, name=f"v{jc}"))
        gam = persist.tile([128, 1], dt.float32)
        gam_ap = gam_d[:, :]
        nc.gpsimd.dma_start(
            out=gam[:],
            in_=bass.AP(
                tensor=gam_ap.tensor, offset=gam_ap.offset, ap=[[0, 128], [1, 1]]
            ),
        )
        shiftb = persist.tile([128, 1], dt.float32)
        nc.vector.memset(shiftb[:], -SHIFT)
        for jc in range(NCH):
            nc.vector.memset(v_ch[jc][:, :, 128:129], 1.0)

        at_s = aw[:, 0, :]
        wv_s = aw[:, 1, :]
        w_ch = [persist.tile([128, 512], dt.float16, name=f"w{k}") for k in range(NWC)]

        epool = ctx.enter_context(tc.tile_pool(name="epsum", bufs=2, space="PSUM"))
        opool = ctx.enter_context(tc.tile_pool(name="opsum", bufs=1, space="PSUM"))
        ptpool = ctx.enter_context(tc.tile_pool(name="ptp", bufs=3))
        spool = ctx.enter_context(tc.tile_pool(name="small", bufs=8))
        osb_pool = ctx.enter_context(tc.tile_pool(name="osb", bufs=3))
        xrpool = ctx.enter_context(tc.tile_pool(name="xrp", bufs=3))
        outpool = ctx.enter_context(tc.tile_pool(name="outp", bufs=3))

        def emit_vprep(jc, pool_tag):
            t = epool.tile([128, 4, 128], dt.float32, tag=pool_tag, name=f"vp{jc}")
            for k in range(4):
                nc.tensor.matmul(
                    t[:, k, :],
                    xt_ch[jc][:, k * 128 : (k + 1) * 128],
                    wv_s,
                    start=True,
                    stop=True,
                )
            nc.vector.tensor_copy(out=v_ch[jc][:, :, 0:128], in_=t[:])

        # ---- prep (PE-warmup + w = A X_i^T + v0/v1), staged through oa banks ----
        wt = opool.tile([128, 512], dt.float32, tag="oa0", name="warmp")
        for r in range(8):
            nc.tensor.matmul(
                wt[:, r * 64 : (r + 1) * 64],
                warm[:],
                warm[:, 0:64],
                start=True,
                stop=True,
            )
        for k in range(NWC):
            t = opool.tile([128, 512], dt.float32, tag=f"oa{k % 2}", name=f"wp{k}")
            nc.tensor.matmul(t[:], at_s, xti_ch[k][:], start=True, stop=True)
            nc.vector.tensor_copy(out=w_ch[k][:], in_=t[:])
        for jc in range(2):
            t = opool.tile([128, 4, 128], dt.float32, tag=f"oa{jc}", name=f"vp{jc}")
            for k in range(4):
                nc.tensor.matmul(
                    t[:, k, :],
                    xt_ch[jc][:, k * 128 : (k + 1) * 128],
                    wv_s,
                    start=True,
                    stop=True,
                )
            nc.vector.tensor_copy(out=v_ch[jc][:, :, 0:128], in_=t[:])

        # ---- main loop ----
        ngroups = (NJB + G - 1) // G
        # v chunks 2..7 are prepped inside ic 0, staggered to when their xT
        # chunk DMA has landed: after attn@v of group jg, prep chunks vprep_at[jg]
        vprep_at = {0: (2, 3), 1: (4,), 2: (5,), 3: (6, 7)}

        def emit_energy(icn, jg, gsz):
            et = epool.tile([128, G, IC], dt.float32, tag="et", name=f"et{icn}_{jg}")
            wsl = w_ch[icn // 2][:, (icn % 2) * IC : (icn % 2 + 1) * IC]
            for g in range(gsz):
                jb = jg * G + g
                nc.tensor.matmul(
                    et[:, g, :],
                    xt_ch[jb // 4][:, (jb % 4) * 128 : (jb % 4 + 1) * 128],
                    wsl,
                    start=True,
                    stop=True,
                )
            return et

        for icn in range(NICH):
            oa = [
                opool.tile([128, 129], dt.float32, tag=f"oa{k}", name=f"oa{k}_{icn}")
                for k in range(NIT)
            ]
            ets = {0: emit_energy(icn, 0, min(G, NJB))}
            for jg in range(ngroups):
                gsz = min(G, NJB - jg * G)
                if jg + 1 < ngroups:
                    # keep PE one group ahead of the exp -> attn@v chain
                    ets[jg + 1] = emit_energy(icn, jg + 1, min(G, NJB - (jg + 1) * G))
                et = ets.pop(jg)
                pt = ptpool.tile(
                    [128, G, IC], dt.bfloat16, tag="pt", name=f"pt{icn}_{jg}"
                )
                nc.scalar.activation(
                    out=pt[:, :gsz, :],
                    in_=et[:, :gsz, :],
                    func=mybir.ActivationFunctionType.Exp,
                    bias=shiftb[:],
                )
                for g in range(gsz):
                    jb = jg * G + g
                    for it in range(NIT):
                        nc.tensor.matmul(
                            oa[it][:],
                            pt[:, g, it * 128 : (it + 1) * 128],
                            v_ch[jb // 4][:, jb % 4, 0:129],
                            start=(jb == 0),
                            stop=(jb == NJB - 1),
                        )
                if icn == 0:
                    for jc in vprep_at.get(jg, ()):
                        emit_vprep(jc, "et")
            for it in range(NIT):
                ti = icn * NIT + it
                # single fast PSUM read frees the accumulator bank quickly
                osb = osb_pool.tile([128, 129], dt.float32, tag="osb", name=f"osb{ti}")
                nc.vector.tensor_copy(out=osb[:], in_=oa[it][:])
                rs = spool.tile([128, 1], dt.float32, tag="rs", name=f"rs{ti}")
                nc.vector.reciprocal(rs[:], osb[:, 128:129])
                nc.vector.tensor_scalar(
                    out=rs[:],
                    in0=rs[:],
                    scalar1=gam[:],
                    scalar2=None,
                    op0=mybir.AluOpType.mult,
                )
                xr = xrpool.tile([128, 128], dt.float32, tag="xr", name=f"xr{ti}")
                nc.gpsimd.dma_start(out=xr[:], in_=xres_d[ti])
                ot = outpool.tile([128, 128], dt.float32, tag="ot", name=f"ot{ti}")
                nc.vector.tensor_scalar(
                    out=ot[:],
                    in0=osb[:, 0:128],
                    scalar1=rs[:],
                    scalar2=None,
                    op0=mybir.AluOpType.mult,
                )
                nc.vector.tensor_tensor(
                    out=ot[:], in0=ot[:], in1=xr[:], op=mybir.AluOpType.add
                )
                nc.sync.dma_start(out=out_d[ti], in_=ot[:])

    nc.finalize()
    return nc


def get_nc():
    if "nc" not in _NC_CACHE:
        _NC_CACHE["nc"] = _build_nc()
    return _NC_CACHE["nc"]


def make_in_maps(x, Wq, Wk, Wv, gamma):
    x = np.asarray(x, dtype=np.float32)
    Wq = np.asarray(Wq, dtype=np.float32)
    Wk = np.asarray(Wk, dtype=np.float32)
    Wv = np.asarray(Wv, dtype=np.float32)
    gamma = np.asarray(gamma, dtype=np.float32)

    xf = x.reshape(B, N, C)
    A = Wq @ Wk.T
    aw = np.ascontiguousarray(
        np.concatenate([A.T.astype(np.float16), Wv.astype(np.float16)], axis=1)
    )  # [128, 256] = [A^T | Wv]
    gam = gamma.reshape(1, 1)

    in_maps = []
    for c in range(NCORES):
        b, ih = c // 2, c % 2
        xT = np.ascontiguousarray(xf[b].T).astype(np.float16)  # [128, 4096]
        sl = slice(ih * NI, (ih + 1) * NI)
        in_maps.append(
            {
                "xT": xT,
                "xTi": np.ascontiguousarray(xT[:, sl]),
                "xres": np.ascontiguousarray(
                    xf[b][sl].reshape(NI // 128, 128, 128)
                ),
                "aw": aw,
                "gam": gam,
            }
        )
    return in_maps


def assemble_out(results):
    outs = [np.asarray(results[c]["out"]).reshape(NI, C) for c in range(NCORES)]
    full = np.stack(
        [np.concatenate([outs[2 * b], outs[2 * b + 1]], axis=0) for b in range(B)]
    )
    return full.reshape(B, Dd, Hh, Ww, C).astype(np.float32)


def kernel(x, Wq, Wk, Wv, gamma):
    from concourse.bass_utils import run_bass_kernel_spmd

    nc = get_nc()
    in_maps = make_in_maps(x, Wq, Wk, Wv, gamma)
    res = run_bass_kernel_spmd(nc, in_maps, core_ids=list(range(NCORES)))
    return assemble_out(res.results)


# revision 13
# speedup vs baseline: 1.1204x; 1.0122x over previous
"""Position-attention kernel for Trainium2 (8 NeuronCores, SPMD data-parallel).

Math (per batch b):
    q = X Wq ; k = X Wk ; v = X Wv          (X = x[b] reshaped [N, C], N=4096, C=128)
    energy[i, j] = k_i . q_j
    attn = softmax(energy, axis=-1)
    out = gamma * (attn @ v) + X

Kernel restructuring:
    energy = X A X^T with A = Wq Wk^T, computed transposed as
    eT[j, i] = sum_c xT[c, j] * w[c, i]  where  w = A @ X_i^T  (tiny prep matmul).
    eT lands in PSUM with j on partitions and is exp'd (softmax is shift/scale
    invariant) by the scalar engine directly into SBUF as bf16 -> already in
    the right layout to be the stationary operand of the attn@v matmul (no
    transposes anywhere). A ones-column appended to v gives the softmax
    denominator for free.

Sharding: 8 cores = (4 batches) x (2 halves of the 4096 output rows).
"""

import numpy as np

B, Dd, Hh, Ww, C = 4, 16, 16, 16, 128
N = Dd * Hh * Ww            # 4096 sequence positions (j)
NCORES = 8
NI = (B * N) // NCORES      # 2048 output rows per core (i)
NJB = N // 128              # 32 j-blocks
G = 6                       # j-blocks per exp group (PSUM: 2*3 + 2 banks = 8)
IC = 256                    # i-chunk (2 accumulator tiles of 128 rows)
NICH = NI // IC             # 8 i-chunks
NIT = IC // 128             # 2 i-tiles per chunk
SHIFT = 32.0                # softmax shift (cancels exactly in normalization)

_NC_CACHE = {}


def _build_nc():
    from contextlib import ExitStack

    import concourse.bacc as bacc
    import concourse.bass as bass
    import concourse.mybir as mybir
    import concourse.tile as tile

    dt = mybir.dt
    nc = bacc.Bacc(target_bir_lowering=False)

    xT_d = nc.declare_dram_parameter("xT", [128, N], dt.float16, isOutput=False)
    xTi_d = nc.declare_dram_parameter("xTi", [128, NI], dt.float16, isOutput=False)
    xres_d = nc.declare_dram_parameter(
        "xres", [NI // 128, 128, 128], dt.float32, isOutput=False
    )
    # weights packed [A^T | Wv] along columns to halve DMA count
    aw_d = nc.declare_dram_parameter("aw", [128, 256], dt.float16, isOutput=False)
    gam_d = nc.declare_dram_parameter("gam", [1, 1], dt.float32, isOutput=False)
    out_d = nc.declare_dram_parameter(
        "out", [NI // 128, 128, 128], dt.float32, isOutput=True
    )

    NCH = N // 512   # 8 column chunks of xT
    NWC = NI // 512  # 4 column chunks of xTi / w
    with tile.TileContext(nc) as tc, ExitStack() as ctx:
        persist = ctx.enter_context(tc.tile_pool(name="persist", bufs=1))

        # warm up the exp table load while DMAs run
        dummy = persist.tile([1, 1], dt.float32)
        nc.vector.memset(dummy[:], 0.0)
        nc.scalar.activation(
            out=dummy[:], in_=dummy[:], func=mybir.ActivationFunctionType.Exp
        )
        # zeroed operand for PE-warmup matmuls
        warm = persist.tile([128, 128], dt.float16)
        nc.vector.memset(warm[:], 0.0)

        # DMA order mirrors the critical path: tiny queue-warmers first, then
        # aw -> xti[0:256] -> xt0/xt1 on the sync queue feed the first energy
        # group; the scalar queue carries the rest.
        qw = persist.tile([1, 4], dt.float16)
        nc.sync.dma_start(out=qw[0:1, 0:2], in_=aw_d[0:1, 0:2])
        nc.scalar.dma_start(out=qw[0:1, 2:4], in_=aw_d[0:1, 2:4])
        aw = persist.tile([128, 2, 128], dt.float16)
        nc.sync.dma_start(out=aw[:], in_=aw_d[:, :])
        xti_ch = [
            persist.tile([128, 512], dt.float16, name=f"xti{k}") for k in range(NWC)
        ]
        nc.sync.dma_start(out=xti_ch[0][:, 0:256], in_=xTi_d[:, 0:256])
        nc.scalar.dma_start(out=xti_ch[0][:, 256:512], in_=xTi_d[:, 256:512])
        for k in range(1, NWC):
            nc.scalar.dma_start(
                out=xti_ch[k][:], in_=xTi_d[:, k * 512 : (k + 1) * 512]
            )
        xt_ch = []
        v_ch = []
        for jc in range(NCH):
            t = persist.tile([128, 512], dt.float16, name=f"xt{jc}")
            eng = nc.sync if jc < 4 else nc.scalar
            eng.dma_start(out=t[:], in_=xT_d[:, jc * 512 : (jc + 1) * 512])
            xt_ch.append(t)
            v_ch.append(persist.tile([128, 4, 132], dt.bfloat16, name=f"v{jc}"))
        gam = persist.tile([128, 1], dt.float32)
        gam_ap = gam_d[:, :]
        nc.gpsimd.dma_start(
            out=gam[:],
            in_=bass.AP(
                tensor=gam_ap.tensor, offset=gam_ap.offset, ap=[[0, 128], [1, 1]]
            ),
        )
        shiftb = persist.tile([128, 1], dt.float32)
        nc.vector.memset(shiftb[:], -SHIFT)
        for jc in range(NCH):
            nc.vector.memset(v_ch[jc][:, :, 128:129], 1.0)

        at_s = aw[:, 0, :]
        wv_s = aw[:, 1, :]
        w_ch = [persist.tile([128, 512], dt.float16, name=f"w{k}") for k in range(NWC)]

        epool = ctx.enter_context(tc.tile_pool(name="epsum", bufs=2, space="PSUM"))
        opool = ctx.enter_context(tc.tile_pool(name="opsum", bufs=1, space="PSUM"))
        ptpool = ctx.enter_context(tc.tile_pool(name="ptp", bufs=3))
        spool = ctx.enter_context(tc.tile_pool(name="small", bufs=8))
        osb_pool = ctx.enter_context(tc.tile_pool(name="osb", bufs=3))
        xrpool = ctx.enter_context(tc.tile_pool(name="xrp", bufs=3))
        outpool = ctx.enter_context(tc.tile_pool(name="outp", bufs=3))

        def emit_vprep(jc, pool_tag):
            t = epool.tile([128, 4, 128], dt.float32, tag=pool_tag, name=f"vp{jc}")
            for k in range(4):
                nc.tensor.matmul(
                    t[:, k, :],
                    xt_ch[jc][:, k * 128 : (k + 1) * 128],
                    wv_s,
                    start=True,
                    stop=True,
                )
            nc.vector.tensor_copy(out=v_ch[jc][:, :, 0:128], in_=t[:])

        # ---- prep (PE-warmup + w = A X_i^T + v0/v1), staged through oa banks ----
        wt = opool.tile([128, 512], dt.float32, tag="oa0", name="warmp")
        for r in range(8):
            nc.tensor.matmul(
                wt[:, r * 64 : (r + 1) * 64],
                warm[:],
                warm[:, 0:64],
                start=True,
                stop=True,
            )
        # w0 in two halves so the first energy group only waits on 64KB of xTi
        t = opool.tile([128, 512], dt.float32, tag="oa0", name="wp0")
        nc.tensor.matmul(t[:, 0:256], at_s, xti_ch[0][:, 0:256], start=True, stop=True)
        nc.vector.tensor_copy(out=w_ch[0][:, 0:256], in_=t[:, 0:256])
        nc.tensor.matmul(
            t[:, 256:512], at_s, xti_ch[0][:, 256:512], start=True, stop=True
        )
        nc.vector.tensor_copy(out=w_ch[0][:, 256:512], in_=t[:, 256:512])
        for k in range(1, NWC):
            t = opool.tile([128, 512], dt.float32, tag=f"oa{k % 2}", name=f"wp{k}")
            nc.tensor.matmul(t[:], at_s, xti_ch[k][:], start=True, stop=True)
            nc.vector.tensor_copy(out=w_ch[k][:], in_=t[:])
        for jc in range(2):
            t = opool.tile([128, 4, 128], dt.float32, tag=f"oa{jc}", name=f"vp{jc}")
            for k in range(4):
                nc.tensor.matmul(
                    t[:, k, :],
                    xt_ch[jc][:, k * 128 : (k + 1) * 128],
                    wv_s,
                    start=True,
                    stop=True,
                )
            nc.vector.tensor_copy(out=v_ch[jc][:, :, 0:128], in_=t[:])

        # ---- main loop ----
        ngroups = (NJB + G - 1) // G
        # v chunks 2..7 are prepped inside ic 0, staggered to when their xT
        # chunk DMA has landed: after attn@v of group jg, prep chunks vprep_at[jg]
        vprep_at = {0: (2, 3), 1: (4,), 2: (5,), 3: (6, 7)}

        def emit_energy(icn, jg, gsz):
            et = epool.tile([128, G, IC], dt.float32, tag="et", name=f"et{icn}_{jg}")
            wsl = w_ch[icn // 2][:, (icn % 2) * IC : (icn % 2 + 1) * IC]
            for g in range(gsz):
                jb = jg * G + g
                nc.tensor.matmul(
                    et[:, g, :],
                    xt_ch[jb // 4][:, (jb % 4) * 128 : (jb % 4 + 1) * 128],
                    wsl,
                    start=True,
                    stop=True,
                )
            return et

        # flat (icn, jg) schedule with one-group PE lookahead ACROSS i-chunk
        # boundaries, so the scalar engine never waits for energy matmuls
        flat = [(icn, jg) for icn in range(NICH) for jg in range(ngroups)]
        ets = {}
        oa_by_ic = {}
        ets[flat[0]] = emit_energy(0, 0, G)
        for fk, (icn, jg) in enumerate(flat):
            gsz = min(G, NJB - jg * G)
            if fk + 1 < len(flat):
                nicn, njg = flat[fk + 1]
                ets[flat[fk + 1]] = emit_energy(
                    nicn, njg, min(G, NJB - njg * G)
                )
            et = ets.pop((icn, jg))
            pt = ptpool.tile(
                [128, G, IC], dt.bfloat16, tag="pt", name=f"pt{icn}_{jg}"
            )
            nc.scalar.activation(
                out=pt[:, :gsz, :],
                in_=et[:, :gsz, :],
                func=mybir.ActivationFunctionType.Exp,
                bias=shiftb[:],
            )
            if jg == 0:
                oa_by_ic[icn] = [
                    opool.tile(
                        [128, 129], dt.float32, tag=f"oa{k}", name=f"oa{k}_{icn}"
                    )
                    for k in range(NIT)
                ]
            oa = oa_by_ic[icn]
            for g in range(gsz):
                jb = jg * G + g
                for it in range(NIT):
                    nc.tensor.matmul(
                        oa[it][:],
                        pt[:, g, it * 128 : (it + 1) * 128],
                        v_ch[jb // 4][:, jb % 4, 0:129],
                        start=(jb == 0),
                        stop=(jb == NJB - 1),
                    )
            if icn == 0:
                for jc in vprep_at.get(jg, ()):
                    emit_vprep(jc, "et")
            if jg != ngroups - 1:
                continue
            # end of i-chunk: normalize, add residual, store
            for it in range(NIT):
                ti = icn * NIT + it
                # single fast PSUM read frees the accumulator bank quickly
                osb = osb_pool.tile([128, 129], dt.float32, tag="osb", name=f"osb{ti}")
                nc.vector.tensor_copy(out=osb[:], in_=oa[it][:])
                rs = spool.tile([128, 1], dt.float32, tag="rs", name=f"rs{ti}")
                nc.vector.reciprocal(rs[:], osb[:, 128:129])
                nc.vector.tensor_scalar(
                    out=rs[:],
                    in0=rs[:],
                    scalar1=gam[:],
                    scalar2=None,
                    op0=mybir.AluOpType.mult,
                )
                xr = xrpool.tile([128, 128], dt.float32, tag="xr", name=f"xr{ti}")
                nc.gpsimd.dma_start(out=xr[:], in_=xres_d[ti])
                ot = outpool.tile([128, 128], dt.float32, tag="ot", name=f"ot{ti}")
                nc.vector.tensor_scalar(
                    out=ot[:],
                    in0=osb[:, 0:128],
                    scalar1=rs[:],
                    scalar2=None,
                    op0=mybir.AluOpType.mult,
                )
                nc.vector.tensor_tensor(
                    out=ot[:], in0=ot[:], in1=xr[:], op=mybir.AluOpType.add
                )
                nc.sync.dma_start(out=out_d[ti], in_=ot[:])

    nc.finalize()
    return nc


def get_nc():
    if "nc" not in _NC_CACHE:
        _NC_CACHE["nc"] = _build_nc()
    return _NC_CACHE["nc"]


def make_in_maps(x, Wq, Wk, Wv, gamma):
    x = np.asarray(x, dtype=np.float32)
    Wq = np.asarray(Wq, dtype=np.float32)
    Wk = np.asarray(Wk, dtype=np.float32)
    Wv = np.asarray(Wv, dtype=np.float32)
    gamma = np.asarray(gamma, dtype=np.float32)

    xf = x.reshape(B, N, C)
    A = Wq @ Wk.T
    aw = np.ascontiguousarray(
        np.concatenate([A.T.astype(np.float16), Wv.astype(np.float16)], axis=1)
    )  # [128, 256] = [A^T | Wv]
    gam = gamma.reshape(1, 1)

    in_maps = []
    for c in range(NCORES):
        b, ih = c // 2, c % 2
        xT = np.ascontiguousarray(xf[b].T).astype(np.float16)  # [128, 4096]
        sl = slice(ih * NI, (ih + 1) * NI)
        in_maps.append(
            {
                "xT": xT,
                "xTi": np.ascontiguousarray(xT[:, sl]),
                "xres": np.ascontiguousarray(
                    xf[b][sl].reshape(NI // 128, 128, 128)
                ),
                "aw": aw,
                "gam": gam,
            }
        )
    return in_maps


def assemble_out(results):
    outs = [np.asarray(results[c]["out"]).reshape(NI, C) for c in range(NCORES)]
    full = np.stack(
        [np.concatenate([outs[2 * b], outs[2 * b + 1]], axis=0) for b in range(B)]
    )
    return full.reshape(B, Dd, Hh, Ww, C).astype(np.float32)


def kernel(x, Wq, Wk, Wv, gamma):
    from concourse.bass_utils import run_bass_kernel_spmd

    nc = get_nc()
    in_maps = make_in_maps(x, Wq, Wk, Wv, gamma)
    res = run_bass_kernel_spmd(nc, in_maps, core_ids=list(range(NCORES)))
    return assemble_out(res.results)


# revision 17
# speedup vs baseline: 1.1447x; 1.0217x over previous
"""Position-attention kernel for Trainium2 (8 NeuronCores, SPMD data-parallel).

Math (per batch b):
    q = X Wq ; k = X Wk ; v = X Wv          (X = x[b] reshaped [N, C], N=4096, C=128)
    energy[i, j] = k_i . q_j
    attn = softmax(energy, axis=-1)
    out = gamma * (attn @ v) + X

Kernel restructuring:
    energy = X A X^T with A = Wq Wk^T, computed transposed as
    eT[j, i] = sum_c xT[c, j] * w[c, i]  where  w = A @ X_i^T  (tiny prep matmul).
    eT lands in PSUM with j on partitions and is exp'd (softmax is shift/scale
    invariant) by the scalar engine directly into SBUF as bf16 -> already in
    the right layout to be the stationary operand of the attn@v matmul (no
    transposes anywhere). A ones-column appended to v gives the softmax
    denominator for free.

Sharding: 8 cores = (4 batches) x (2 halves of the 4096 output rows).
"""

import numpy as np

B, Dd, Hh, Ww, C = 4, 16, 16, 16, 128
N = Dd * Hh * Ww            # 4096 sequence positions (j)
NCORES = 8
NI = (B * N) // NCORES      # 2048 output rows per core (i)
NJB = N // 128              # 32 j-blocks
G = 6                       # j-blocks per exp group (PSUM: 2*3 + 2 banks = 8)
IC = 256                    # i-chunk (2 accumulator tiles of 128 rows)
NICH = NI // IC             # 8 i-chunks
NIT = IC // 128             # 2 i-tiles per chunk
SHIFT = 32.0                # softmax shift (cancels exactly in normalization)

_NC_CACHE = {}


def _build_nc():
    from contextlib import ExitStack

    import concourse.bacc as bacc
    import concourse.bass as bass
    import concourse.mybir as mybir
    import concourse.tile as tile

    dt = mybir.dt
    nc = bacc.Bacc(target_bir_lowering=False)

    xT_d = nc.declare_dram_parameter("xT", [128, N], dt.float16, isOutput=False)
    xTi_d = nc.declare_dram_parameter("xTi", [128, NI], dt.float16, isOutput=False)
    xres_d = nc.declare_dram_parameter(
        "xres", [NI // 128, 128, 128], dt.float32, isOutput=False
    )
    # weights packed [A^T | Wv] along columns to halve DMA count
    aw_d = nc.declare_dram_parameter("aw", [128, 256], dt.float16, isOutput=False)
    gam_d = nc.declare_dram_parameter("gam", [1, 1], dt.float32, isOutput=False)
    out_d = nc.declare_dram_parameter(
        "out", [NI // 128, 128, 128], dt.float32, isOutput=True
    )

    NCH = N // 512   # 8 column chunks of xT
    NWC = NI // 512  # 4 column chunks of xTi / w
    with tile.TileContext(nc) as tc, ExitStack() as ctx:
        persist = ctx.enter_context(tc.tile_pool(name="persist", bufs=1))

        # warm up the exp table load while DMAs run
        dummy = persist.tile([1, 1], dt.float32)
        nc.vector.memset(dummy[:], 0.0)
        nc.scalar.activation(
            out=dummy[:], in_=dummy[:], func=mybir.ActivationFunctionType.Exp
        )
        # zeroed operand for PE-warmup matmuls
        warm = persist.tile([128, 128], dt.float16)
        nc.vector.memset(warm[:], 0.0)

        # DMA order mirrors the critical path: tiny queue-warmers first, then
        # aw -> xti[0:256] -> xt0/xt1 on the sync queue feed the first energy
        # group; the scalar queue carries the rest.
        qw = persist.tile([1, 4], dt.float16)
        nc.sync.dma_start(out=qw[0:1, 0:2], in_=aw_d[0:1, 0:2])
        nc.scalar.dma_start(out=qw[0:1, 2:4], in_=aw_d[0:1, 2:4])
        aw = persist.tile([128, 2, 128], dt.float16)
        xti_ch = [
            persist.tile([128, 512], dt.float16, name=f"xti{k}") for k in range(NWC)
        ]
        xt_ch = [
            persist.tile([128, 512], dt.float16, name=f"xt{jc}") for jc in range(NCH)
        ]
        v_ch = [
            persist.tile([128, 4, 132], dt.bfloat16, name=f"v{jc}")
            for jc in range(NCH)
        ]

        def ld_xt(eng, jc):
            eng.dma_start(out=xt_ch[jc][:], in_=xT_d[:, jc * 512 : (jc + 1) * 512])

        # sync queue: aw -> xt0 -> xt2 -> xt3 -> xti[256:512] -> xt5
        # scalar queue: xti[0:256] -> xt1 -> xti1..3 -> xt4 -> xt6 -> xt7
        nc.sync.dma_start(out=aw[:], in_=aw_d[:, :])
        nc.scalar.dma_start(out=xti_ch[0][:, 0:256], in_=xTi_d[:, 0:256])
        ld_xt(nc.sync, 0)
        ld_xt(nc.scalar, 1)
        ld_xt(nc.sync, 2)
        ld_xt(nc.sync, 3)
        nc.sync.dma_start(out=xti_ch[0][:, 256:512], in_=xTi_d[:, 256:512])
        for k in range(1, NWC):
            nc.scalar.dma_start(
                out=xti_ch[k][:], in_=xTi_d[:, k * 512 : (k + 1) * 512]
            )
        ld_xt(nc.scalar, 4)
        ld_xt(nc.sync, 5)
        ld_xt(nc.scalar, 6)
        ld_xt(nc.scalar, 7)
        gam = persist.tile([128, 1], dt.float32)
        gam_ap = gam_d[:, :]
        nc.gpsimd.dma_start(
            out=gam[:],
            in_=bass.AP(
                tensor=gam_ap.tensor, offset=gam_ap.offset, ap=[[0, 128], [1, 1]]
            ),
        )
        shiftb = persist.tile([128, 1], dt.float32)
        nc.vector.memset(shiftb[:], -SHIFT)
        for jc in range(NCH):
            nc.vector.memset(v_ch[jc][:, :, 128:129], 1.0)

        at_s = aw[:, 0, :]
        wv_s = aw[:, 1, :]
        w_ch = [persist.tile([128, 512], dt.float16, name=f"w{k}") for k in range(NWC)]

        epool = ctx.enter_context(tc.tile_pool(name="epsum", bufs=2, space="PSUM"))
        opool = ctx.enter_context(tc.tile_pool(name="opsum", bufs=1, space="PSUM"))
        ptpool = ctx.enter_context(tc.tile_pool(name="ptp", bufs=3))
        spool = ctx.enter_context(tc.tile_pool(name="small", bufs=8))
        osb_pool = ctx.enter_context(tc.tile_pool(name="osb", bufs=3))
        xrpool = ctx.enter_context(tc.tile_pool(name="xrp", bufs=3))
        outpool = ctx.enter_context(tc.tile_pool(name="outp", bufs=3))

        def emit_vprep(jc, pool_tag):
            t = epool.tile([128, 4, 128], dt.float32, tag=pool_tag, name=f"vp{jc}")
            for k in range(4):
                nc.tensor.matmul(
                    t[:, k, :],
                    xt_ch[jc][:, k * 128 : (k + 1) * 128],
                    wv_s,
                    start=True,
                    stop=True,
                )
            nc.vector.tensor_copy(out=v_ch[jc][:, :, 0:128], in_=t[:])

        # ---- prep (PE-warmup + w = A X_i^T + v0/v1), staged through oa banks ----
        wt = opool.tile([128, 512], dt.float32, tag="oa0", name="warmp")
        for r in range(8):
            nc.tensor.matmul(
                wt[:, r * 64 : (r + 1) * 64],
                warm[:],
                warm[:, 0:64],
                start=True,
                stop=True,
            )
        # w0 in two halves so the first energy group only waits on 64KB of xTi
        t = opool.tile([128, 512], dt.float32, tag="oa0", name="wp0")
        nc.tensor.matmul(t[:, 0:256], at_s, xti_ch[0][:, 0:256], start=True, stop=True)
        nc.vector.tensor_copy(out=w_ch[0][:, 0:256], in_=t[:, 0:256])
        nc.tensor.matmul(
            t[:, 256:512], at_s, xti_ch[0][:, 256:512], start=True, stop=True
        )
        nc.vector.tensor_copy(out=w_ch[0][:, 256:512], in_=t[:, 256:512])
        for k in range(1, NWC):
            t = opool.tile([128, 512], dt.float32, tag=f"oa{k % 2}", name=f"wp{k}")
            nc.tensor.matmul(t[:], at_s, xti_ch[k][:], start=True, stop=True)
            nc.vector.tensor_copy(out=w_ch[k][:], in_=t[:])
        for jc in range(4):
            t = opool.tile(
                [128, 4, 128], dt.float32, tag=f"oa{jc % 2}", name=f"vp{jc}"
            )
            for k in range(4):
                nc.tensor.matmul(
                    t[:, k, :],
                    xt_ch[jc][:, k * 128 : (k + 1) * 128],
                    wv_s,
                    start=True,
                    stop=True,
                )
            nc.vector.tensor_copy(out=v_ch[jc][:, :, 0:128], in_=t[:])

        # ---- main loop ----
        ngroups = (NJB + G - 1) // G
        # v chunks 4..7 are prepped inside ic 0, staggered to when their xT
        # chunk DMA has landed: after attn@v of group jg, prep chunks vprep_at[jg]
        vprep_at = {1: (4,), 2: (5,), 3: (6, 7)}

        def emit_energy(icn, jg, gsz):
            et = epool.tile([128, G, IC], dt.float32, tag="et", name=f"et{icn}_{jg}")
            wsl = w_ch[icn // 2][:, (icn % 2) * IC : (icn % 2 + 1) * IC]
            for g in range(gsz):
                jb = jg * G + g
                nc.tensor.matmul(
                    et[:, g, :],
                    xt_ch[jb // 4][:, (jb % 4) * 128 : (jb % 4 + 1) * 128],
                    wsl,
                    start=True,
                    stop=True,
                )
            return et

        # flat (icn, jg) schedule with one-group PE lookahead ACROSS i-chunk
        # boundaries, so the scalar engine never waits for energy matmuls
        flat = [(icn, jg) for icn in range(NICH) for jg in range(ngroups)]
        ets = {}
        oa_by_ic = {}
        ets[flat[0]] = emit_energy(0, 0, G)
        for fk, (icn, jg) in enumerate(flat):
            gsz = min(G, NJB - jg * G)
            if fk + 1 < len(flat):
                nicn, njg = flat[fk + 1]
                ets[flat[fk + 1]] = emit_energy(
                    nicn, njg, min(G, NJB - njg * G)
                )
            et = ets.pop((icn, jg))
            pt = ptpool.tile(
                [128, G, IC], dt.bfloat16, tag="pt", name=f"pt{icn}_{jg}"
            )
            nc.scalar.activation(
                out=pt[:, :gsz, :],
                in_=et[:, :gsz, :],
                func=mybir.ActivationFunctionType.Exp,
                bias=shiftb[:],
            )
            if jg == 0:
                oa_by_ic[icn] = [
                    opool.tile(
                        [128, 129], dt.float32, tag=f"oa{k}", name=f"oa{k}_{icn}"
                    )
                    for k in range(NIT)
                ]
            oa = oa_by_ic[icn]
            for g in range(gsz):
                jb = jg * G + g
                for it in range(NIT):
                    nc.tensor.matmul(
                        oa[it][:],
                        pt[:, g, it * 128 : (it + 1) * 128],
                        v_ch[jb // 4][:, jb % 4, 0:129],
                        start=(jb == 0),
                        stop=(jb == NJB - 1),
                    )
            if icn == 0:
                for jc in vprep_at.get(jg, ()):
                    emit_vprep(jc, "et")
            if jg != ngroups - 1:
                continue
            # end of i-chunk: normalize, add residual, store
            for it in range(NIT):
                ti = icn * NIT + it
                # single fast PSUM read frees the accumulator bank quickly
                osb = osb_pool.tile([128, 129], dt.float32, tag="osb", name=f"osb{ti}")
                nc.vector.tensor_copy(out=osb[:], in_=oa[it][:])
                rs = spool.tile([128, 1], dt.float32, tag="rs", name=f"rs{ti}")
                nc.vector.reciprocal(rs[:], osb[:, 128:129])
                nc.vector.tensor_scalar(
                    out=rs[:],
                    in0=rs[:],
                    scalar1=gam[:],
                    scalar2=None,
                    op0=mybir.AluOpType.mult,
                )
                xr = xrpool.tile([128, 128], dt.float32, tag="xr", name=f"xr{ti}")
                nc.sync.dma_start(out=xr[:], in_=xres_d[ti])
                ot = outpool.tile([128, 128], dt.float32, tag="ot", name=f"ot{ti}")
                nc.vector.tensor_scalar(
                    out=ot[:],
                    in0=osb[:, 0:128],
                    scalar1=rs[:],
                    scalar2=None,
                    op0=mybir.AluOpType.mult,
                )
                nc.vector.tensor_tensor(
                    out=ot[:], in0=ot[:], in1=xr[:], op=mybir.AluOpType.add
                )
                nc.sync.dma_start(out=out_d[ti], in_=ot[:])

    nc.finalize()
    return nc


def get_nc():
    if "nc" not in _NC_CACHE:
        _NC_CACHE["nc"] = _build_nc()
    return _NC_CACHE["nc"]


def make_in_maps(x, Wq, Wk, Wv, gamma):
    x = np.asarray(x, dtype=np.float32)
    Wq = np.asarray(Wq, dtype=np.float32)
    Wk = np.asarray(Wk, dtype=np.float32)
    Wv = np.asarray(Wv, dtype=np.float32)
    gamma = np.asarray(gamma, dtype=np.float32)

    xf = x.reshape(B, N, C)
    A = Wq @ Wk.T
    aw = np.ascontiguousarray(
        np.concatenate([A.T.astype(np.float16), Wv.astype(np.float16)], axis=1)
    )  # [128, 256] = [A^T | Wv]
    gam = gamma.reshape(1, 1)

    in_maps = []
    for c in range(NCORES):
        b, ih = c // 2, c % 2
        xT = np.ascontiguousarray(xf[b].T).astype(np.float16)  # [128, 4096]
        sl = slice(ih * NI, (ih + 1) * NI)
        in_maps.append(
            {
                "xT": xT,
                "xTi": np.ascontiguousarray(xT[:, sl]),
                "xres": np.ascontiguousarray(
                    xf[b][sl].reshape(NI // 128, 128, 128)
                ),
                "aw": aw,
                "gam": gam,
            }
        )
    return in_maps


def assemble_out(results):
    outs = [np.asarray(results[c]["out"]).reshape(NI, C) for c in range(NCORES)]
    full = np.stack(
        [np.concatenate([outs[2 * b], outs[2 * b + 1]], axis=0) for b in range(B)]
    )
    return full.reshape(B, Dd, Hh, Ww, C).astype(np.float32)


def kernel(x, Wq, Wk, Wv, gamma):
    from concourse.bass_utils import run_bass_kernel_spmd

    nc = get_nc()
    in_maps = make_in_maps(x, Wq, Wk, Wv, gamma)
    res = run_bass_kernel_spmd(nc, in_maps, core_ids=list(range(NCORES)))
    return assemble_out(res.results)


# revision 25
# speedup vs baseline: 1.1464x; 1.0015x over previous
"""Position-attention kernel for Trainium2 (8 NeuronCores, SPMD data-parallel).

Math (per batch b):
    q = X Wq ; k = X Wk ; v = X Wv          (X = x[b] reshaped [N, C], N=4096, C=128)
    energy[i, j] = k_i . q_j
    attn = softmax(energy, axis=-1)
    out = gamma * (attn @ v) + X

Kernel restructuring:
    energy = X A X^T with A = Wq Wk^T, computed transposed as
    eT[j, i] = sum_c xT[c, j] * w[c, i]  where  w = A @ X_i^T  (tiny prep matmul).
    eT lands in PSUM with j on partitions and is exp'd (softmax is shift/scale
    invariant) by the scalar engine directly into SBUF as bf16 -> already in
    the right layout to be the stationary operand of the attn@v matmul (no
    transposes anywhere). A ones-column appended to v gives the softmax
    denominator for free.

Sharding: 8 cores = (4 batches) x (2 halves of the 4096 output rows).
"""

import numpy as np

B, Dd, Hh, Ww, C = 4, 16, 16, 16, 128
N = Dd * Hh * Ww            # 4096 sequence positions (j)
NCORES = 8
NI = (B * N) // NCORES      # 2048 output rows per core (i)
NJB = N // 128              # 32 j-blocks
G = 6                       # j-blocks per exp group (PSUM: 2*3 + 2 banks = 8)
IC = 256                    # i-chunk (2 accumulator tiles of 128 rows)
NICH = NI // IC             # 8 i-chunks
NIT = IC // 128             # 2 i-tiles per chunk
SHIFT = 32.0                # softmax shift (cancels exactly in normalization)

_NC_CACHE = {}


def _build_nc():
    from contextlib import ExitStack

    import concourse.bacc as bacc
    import concourse.bass as bass
    import concourse.mybir as mybir
    import concourse.tile as tile

    dt = mybir.dt
    nc = bacc.Bacc(target_bir_lowering=False)

    xT_d = nc.declare_dram_parameter("xT", [128, N], dt.float16, isOutput=False)
    xres_d = nc.declare_dram_parameter(
        "xres", [NI // 128, 128, 128], dt.float32, isOutput=False
    )
    # weights packed [A^T | Wv] along columns to halve DMA count
    aw_d = nc.declare_dram_parameter("aw", [128, 256], dt.float16, isOutput=False)
    gam_d = nc.declare_dram_parameter("gam", [1, 1], dt.float32, isOutput=False)
    out_d = nc.declare_dram_parameter(
        "out", [NI // 128, 128, 128], dt.float32, isOutput=True
    )

    NCH = N // 512   # 8 column chunks of xT
    NWC = NI // 512  # 4 column chunks of xTi / w
    with tile.TileContext(nc) as tc, ExitStack() as ctx:
        persist = ctx.enter_context(tc.tile_pool(name="persist", bufs=1))

        # warm up the exp table load while DMAs run
        dummy = persist.tile([1, 1], dt.float32)
        nc.vector.memset(dummy[:], 0.0)
        nc.scalar.activation(
            out=dummy[:], in_=dummy[:], func=mybir.ActivationFunctionType.Exp
        )
        # zeroed operand for PE-warmup matmuls
        warm = persist.tile([128, 128], dt.float16)
        nc.vector.memset(warm[:], 0.0)

        # DMA order mirrors the critical path: tiny queue-warmers first, then
        # aw -> xti[0:256] -> xt0/xt1 on the sync queue feed the first energy
        # group; the scalar queue carries the rest.
        qw = persist.tile([1, 4], dt.float16)
        nc.sync.dma_start(out=qw[0:1, 0:2], in_=aw_d[0:1, 0:2])
        nc.scalar.dma_start(out=qw[0:1, 2:4], in_=aw_d[0:1, 2:4])
        aw = persist.tile([128, 2, 128], dt.float16)
        xt_ch = [
            persist.tile([128, 512], dt.float16, name=f"xt{jc}") for jc in range(NCH)
        ]
        v_ch = [
            persist.tile([128, 4, 132], dt.bfloat16, name=f"v{jc}")
            for jc in range(NCH)
        ]

        def ld_xt(eng, jc):
            eng.dma_start(out=xt_ch[jc][:], in_=xT_d[:, jc * 512 : (jc + 1) * 512])

        # x is j-rotated on the host so each core's own i-rows are chunks 0-3;
        # interleave the two HW queues so the first-group inputs land first
        nc.sync.dma_start(out=aw[:], in_=aw_d[:, :])
        for jc in range(NCH):
            ld_xt(nc.sync if jc % 2 == 0 else nc.scalar, jc)
        gam = persist.tile([128, 1], dt.float32)
        gam_ap = gam_d[:, :]
        nc.gpsimd.dma_start(
            out=gam[:],
            in_=bass.AP(
                tensor=gam_ap.tensor, offset=gam_ap.offset, ap=[[0, 128], [1, 1]]
            ),
        )
        shiftb = persist.tile([128, 1], dt.float32)
        nc.vector.memset(shiftb[:], -SHIFT)
        for jc in range(NCH):
            nc.vector.memset(v_ch[jc][:, :, 128:129], 1.0)

        at_s = aw[:, 0, :]
        wv_s = aw[:, 1, :]
        w_ch = [persist.tile([128, 512], dt.float16, name=f"w{k}") for k in range(NWC)]

        epool = ctx.enter_context(tc.tile_pool(name="epsum", bufs=2, space="PSUM"))
        opool = ctx.enter_context(tc.tile_pool(name="opsum", bufs=1, space="PSUM"))
        ptpool = ctx.enter_context(tc.tile_pool(name="ptp", bufs=4))
        spool = ctx.enter_context(tc.tile_pool(name="small", bufs=8))
        osb_pool = ctx.enter_context(tc.tile_pool(name="osb", bufs=3))
        xrpool = ctx.enter_context(tc.tile_pool(name="xrp", bufs=3))
        outpool = ctx.enter_context(tc.tile_pool(name="outp", bufs=3))

        def emit_vprep2(jc, pool_tag):
            # two v chunks through one et-pool slot to halve rotation stalls
            t = epool.tile([128, 8, 128], dt.float32, tag=pool_tag, name=f"vp{jc}")
            for h in range(2):
                for k in range(4):
                    nc.tensor.matmul(
                        t[:, h * 4 + k, :],
                        xt_ch[jc + h][:, k * 128 : (k + 1) * 128],
                        wv_s,
                        start=True,
                        stop=True,
                    )
                nc.vector.tensor_copy(
                    out=v_ch[jc + h][:, :, 0:128], in_=t[:, h * 4 : h * 4 + 4, :]
                )

        # ---- prep (PE-warmup + w = A X_i^T + v0/v1), staged through oa banks ----
        wt = opool.tile([128, 512], dt.float32, tag="oa0", name="warmp")
        for r in range(8):
            nc.tensor.matmul(
                wt[:, r * 64 : (r + 1) * 64],
                warm[:],
                warm[:, 0:64],
                start=True,
                stop=True,
            )
        # w0 in two halves so the first energy group only waits on xt0
        t = opool.tile([128, 512], dt.float32, tag="oa0", name="wp0")
        nc.tensor.matmul(t[:, 0:256], at_s, xt_ch[0][:, 0:256], start=True, stop=True)
        nc.vector.tensor_copy(out=w_ch[0][:, 0:256], in_=t[:, 0:256])
        nc.tensor.matmul(
            t[:, 256:512], at_s, xt_ch[0][:, 256:512], start=True, stop=True
        )
        nc.vector.tensor_copy(out=w_ch[0][:, 256:512], in_=t[:, 256:512])
        for k in range(1, NWC):
            t = opool.tile([128, 512], dt.float32, tag=f"oa{k % 2}", name=f"wp{k}")
            nc.tensor.matmul(t[:], at_s, xt_ch[k][:], start=True, stop=True)
            nc.vector.tensor_copy(out=w_ch[k][:], in_=t[:])
        for jc in range(4):
            t = opool.tile(
                [128, 4, 128], dt.float32, tag=f"oa{jc % 2}", name=f"vp{jc}"
            )
            for k in range(4):
                nc.tensor.matmul(
                    t[:, k, :],
                    xt_ch[jc][:, k * 128 : (k + 1) * 128],
                    wv_s,
                    start=True,
                    stop=True,
                )
            nc.vector.tensor_copy(out=v_ch[jc][:, :, 0:128], in_=t[:])

        # ---- main loop ----
        ngroups = (NJB + G - 1) // G
        # v chunks 4..7 are prepped inside ic 0, staggered to when their xT
        # chunk DMA has landed: after attn@v of group jg, prep chunks vprep_at[jg]
        vprep_at = {0: (4,), 1: (6,)}

        def emit_energy(icn, jg, gsz):
            et = epool.tile([128, G, IC], dt.float32, tag="et", name=f"et{icn}_{jg}")
            wsl = w_ch[icn // 2][:, (icn % 2) * IC : (icn % 2 + 1) * IC]
            for g in range(gsz):
                jb = jg * G + g
                nc.tensor.matmul(
                    et[:, g, :],
                    xt_ch[jb // 4][:, (jb % 4) * 128 : (jb % 4 + 1) * 128],
                    wsl,
                    start=True,
                    stop=True,
                )
            return et

        # flat (icn, jg) schedule with one-group PE lookahead ACROSS i-chunk
        # boundaries, so the scalar engine never waits for energy matmuls
        flat = [(icn, jg) for icn in range(NICH) for jg in range(ngroups)]
        ets = {}
        oa_by_ic = {}
        ets[flat[0]] = emit_energy(0, 0, G)
        for fk, (icn, jg) in enumerate(flat):
            gsz = min(G, NJB - jg * G)
            if fk + 1 < len(flat):
                nicn, njg = flat[fk + 1]
                ets[flat[fk + 1]] = emit_energy(
                    nicn, njg, min(G, NJB - njg * G)
                )
            et = ets.pop((icn, jg))
            pt = ptpool.tile(
                [128, G, IC], dt.bfloat16, tag="pt", name=f"pt{icn}_{jg}"
            )
            nc.scalar.activation(
                out=pt[:, :gsz, :],
                in_=et[:, :gsz, :],
                func=mybir.ActivationFunctionType.Exp,
                bias=shiftb[:],
            )
            if jg == 0:
                oa_by_ic[icn] = [
                    opool.tile(
                        [128, 129], dt.float32, tag=f"oa{k}", name=f"oa{k}_{icn}"
                    )
                    for k in range(NIT)
                ]
            oa = oa_by_ic[icn]
            for g in range(gsz):
                jb = jg * G + g
                for it in range(NIT):
                    nc.tensor.matmul(
                        oa[it][:],
                        pt[:, g, it * 128 : (it + 1) * 128],
                        v_ch[jb // 4][:, jb % 4, 0:129],
                        start=(jb == 0),
                        stop=(jb == NJB - 1),
                    )
            if icn == 0:
                for jc in vprep_at.get(jg, ()):
                    emit_vprep2(jc, "et")
            if jg != ngroups - 1:
                continue
            # end of i-chunk: normalize, add residual, store
            for it in range(NIT):
                ti = icn * NIT + it
                # single fast PSUM read frees the accumulator bank quickly
                osb = osb_pool.tile([128, 129], dt.float32, tag="osb", name=f"osb{ti}")
                nc.vector.tensor_copy(out=osb[:], in_=oa[it][:])
                rs = spool.tile([128, 1], dt.float32, tag="rs", name=f"rs{ti}")
                nc.vector.reciprocal(rs[:], osb[:, 128:129])
                nc.vector.tensor_scalar(
                    out=rs[:],
                    in0=rs[:],
                    scalar1=gam[:],
                    scalar2=None,
                    op0=mybir.AluOpType.mult,
                )
                xr = xrpool.tile([128, 128], dt.float32, tag="xr", name=f"xr{ti}")
                nc.sync.dma_start(out=xr[:], in_=xres_d[ti])
                ot = outpool.tile([128, 128], dt.float32, tag="ot", name=f"ot{ti}")
                nc.vector.tensor_scalar(
                    out=ot[:],
                    in0=osb[:, 0:128],
                    scalar1=rs[:],
                    scalar2=None,
                    op0=mybir.AluOpType.mult,
                )
                nc.vector.tensor_tensor(
                    out=ot[:], in0=ot[:], in1=xr[:], op=mybir.AluOpType.add
                )
                nc.sync.dma_start(out=out_d[ti], in_=ot[:])

    nc.finalize()
    return nc


def get_nc():
    if "nc" not in _NC_CACHE:
        _NC_CACHE["nc"] = _build_nc()
    return _NC_CACHE["nc"]


def make_in_maps(x, Wq, Wk, Wv, gamma):
    x = np.asarray(x, dtype=np.float32)
    Wq = np.asarray(Wq, dtype=np.float32)
    Wk = np.asarray(Wk, dtype=np.float32)
    Wv = np.asarray(Wv, dtype=np.float32)
    gamma = np.asarray(gamma, dtype=np.float32)

    xf = x.reshape(B, N, C)
    A = Wq @ Wk.T
    aw = np.ascontiguousarray(
        np.concatenate([A.T.astype(np.float16), Wv.astype(np.float16)], axis=1)
    )  # [128, 256] = [A^T | Wv]
    gam = gamma.reshape(1, 1)

    in_maps = []
    for c in range(NCORES):
        b, ih = c // 2, c % 2
        xT = xf[b].T.astype(np.float16)  # [128, 4096]
        # rotate the j-order so this core's own i-rows are columns 0:NI
        # (softmax sums over j, so any j-order works as long as v matches)
        xTr = np.ascontiguousarray(np.roll(xT, -ih * NI, axis=1))
        sl = slice(ih * NI, (ih + 1) * NI)
        in_maps.append(
            {
                "xT": xTr,
                "xres": np.ascontiguousarray(
                    xf[b][sl].reshape(NI // 128, 128, 128)
                ),
                "aw": aw,
                "gam": gam,
            }
        )
    return in_maps


def assemble_out(results):
    outs = [np.asarray(results[c]["out"]).reshape(NI, C) for c in range(NCORES)]
    full = np.stack(
        [np.concatenate([outs[2 * b], outs[2 * b + 1]], axis=0) for b in range(B)]
    )
    return full.reshape(B, Dd, Hh, Ww, C).astype(np.float32)


def kernel(x, Wq, Wk, Wv, gamma):
    from concourse.bass_utils import run_bass_kernel_spmd

    nc = get_nc()
    in_maps = make_in_maps(x, Wq, Wk, Wv, gamma)
    res = run_bass_kernel_spmd(nc, in_maps, core_ids=list(range(NCORES)))
    return assemble_out(res.results)


# revision 29
# speedup vs baseline: 1.1600x; 1.0119x over previous
"""Position-attention kernel for Trainium2 (8 NeuronCores, SPMD data-parallel).

Math (per batch b):
    q = X Wq ; k = X Wk ; v = X Wv          (X = x[b] reshaped [N, C], N=4096, C=128)
    energy[i, j] = k_i . q_j
    attn = softmax(energy, axis=-1)
    out = gamma * (attn @ v) + X

Kernel restructuring:
    energy = X A X^T with A = Wq Wk^T, computed transposed as
    eT[j, i] = sum_c xT[c, j] * w[c, i]  where  w = A @ X_i^T  (tiny prep matmul).
    eT lands in PSUM with j on partitions and is exp'd (softmax is shift/scale
    invariant) by the scalar engine directly into SBUF as bf16 -> already in
    the right layout to be the stationary operand of the attn@v matmul (no
    transposes anywhere). A ones-column appended to v gives the softmax
    denominator for free.

Sharding: 8 cores = (4 batches) x (2 halves of the 4096 output rows).
"""

import numpy as np

B, Dd, Hh, Ww, C = 4, 16, 16, 16, 128
N = Dd * Hh * Ww            # 4096 sequence positions (j)
NCORES = 8
NI = (B * N) // NCORES      # 2048 output rows per core (i)
NJB = N // 128              # 32 j-blocks
G = 6                       # j-blocks per exp group (PSUM: 2*3 + 2 banks = 8)
IC = 256                    # i-chunk (2 accumulator tiles of 128 rows)
NICH = NI // IC             # 8 i-chunks
NIT = IC // 128             # 2 i-tiles per chunk
SHIFT = 32.0                # softmax shift (cancels exactly in normalization)

_NC_CACHE = {}


def _build_nc():
    from contextlib import ExitStack

    import concourse.bacc as bacc
    import concourse.bass as bass
    import concourse.mybir as mybir
    import concourse.tile as tile

    dt = mybir.dt
    nc = bacc.Bacc(target_bir_lowering=False)

    xT_d = nc.declare_dram_parameter("xT", [128, N], dt.float16, isOutput=False)
    xres_d = nc.declare_dram_parameter(
        "xres", [NI // 128, 128, 128], dt.float32, isOutput=False
    )
    # weights packed [A^T | Wv] along columns to halve DMA count
    aw_d = nc.declare_dram_parameter("aw", [128, 256], dt.float16, isOutput=False)
    gam_d = nc.declare_dram_parameter("gam", [1, 1], dt.float32, isOutput=False)
    out_d = nc.declare_dram_parameter(
        "out", [NI // 128, 128, 128], dt.float32, isOutput=True
    )

    NCH = N // 512   # 8 column chunks of xT
    NWC = NI // 512  # 4 column chunks of xTi / w
    with tile.TileContext(nc) as tc, ExitStack() as ctx:
        persist = ctx.enter_context(tc.tile_pool(name="persist", bufs=1))

        # warm up the exp table load while DMAs run
        dummy = persist.tile([1, 1], dt.float32)
        nc.vector.memset(dummy[:], 0.0)
        nc.scalar.activation(
            out=dummy[:], in_=dummy[:], func=mybir.ActivationFunctionType.Exp
        )
        # zeroed operand for PE-warmup matmuls
        warm = persist.tile([128, 128], dt.float16)
        nc.vector.memset(warm[:], 0.0)

        # DMA order mirrors the critical path: tiny queue-warmers first, then
        # aw -> xti[0:256] -> xt0/xt1 on the sync queue feed the first energy
        # group; the scalar queue carries the rest.
        qw = persist.tile([1, 4], dt.float16)
        nc.sync.dma_start(out=qw[0:1, 0:2], in_=aw_d[0:1, 0:2])
        nc.scalar.dma_start(out=qw[0:1, 2:4], in_=aw_d[0:1, 2:4])
        aw = persist.tile([128, 2, 128], dt.float16)
        xt_ch = [
            persist.tile([128, 512], dt.float16, name=f"xt{jc}") for jc in range(NCH)
        ]
        v_ch = [
            persist.tile([128, 4, 132], dt.bfloat16, name=f"v{jc}")
            for jc in range(NCH)
        ]

        def ld_xt(eng, jc):
            eng.dma_start(out=xt_ch[jc][:], in_=xT_d[:, jc * 512 : (jc + 1) * 512])

        # x is j-rotated on the host so each core's own i-rows are chunks 0-3;
        # interleave the two HW queues so the first-group inputs land first
        nc.sync.dma_start(out=aw[:], in_=aw_d[:, :])
        ld_xt(nc.scalar, 0)
        for jc in range(1, NCH):
            ld_xt(nc.sync if jc % 2 == 1 else nc.scalar, jc)
        gam = persist.tile([128, 1], dt.float32)
        gam_ap = gam_d[:, :]
        nc.gpsimd.dma_start(
            out=gam[:],
            in_=bass.AP(
                tensor=gam_ap.tensor, offset=gam_ap.offset, ap=[[0, 128], [1, 1]]
            ),
        )
        shiftb = persist.tile([128, 1], dt.float32)
        nc.vector.memset(shiftb[:], -SHIFT)
        for jc in range(NCH):
            nc.vector.memset(v_ch[jc][:, :, 128:129], 1.0)

        at_s = aw[:, 0, :]
        wv_s = aw[:, 1, :]
        w_ch = [persist.tile([128, 512], dt.float16, name=f"w{k}") for k in range(NWC)]

        epool = ctx.enter_context(tc.tile_pool(name="epsum", bufs=2, space="PSUM"))
        opool = ctx.enter_context(tc.tile_pool(name="opsum", bufs=1, space="PSUM"))
        ptpool = ctx.enter_context(tc.tile_pool(name="ptp", bufs=8))
        spool = ctx.enter_context(tc.tile_pool(name="small", bufs=8))
        osb_pool = ctx.enter_context(tc.tile_pool(name="osb", bufs=3))
        xrpool = ctx.enter_context(tc.tile_pool(name="xrp", bufs=3))
        outpool = ctx.enter_context(tc.tile_pool(name="outp", bufs=3))

        def emit_vprep(jc, tag):
            t = opool.tile([128, 4, 128], dt.float32, tag=tag, name=f"vp{jc}")
            for k in range(4):
                nc.tensor.matmul(
                    t[:, k, :],
                    xt_ch[jc][:, k * 128 : (k + 1) * 128],
                    wv_s,
                    start=True,
                    stop=True,
                )
            nc.vector.tensor_copy(out=v_ch[jc][:, :, 0:128], in_=t[:])

        def emit_wprep(k, tag, half=None):
            if half is None:
                t = opool.tile([128, 512], dt.float32, tag=tag, name=f"wp{k}")
                nc.tensor.matmul(t[:], at_s, xt_ch[k][:], start=True, stop=True)
                nc.vector.tensor_copy(out=w_ch[k][:], in_=t[:])
            else:
                sl = slice(half * 256, half * 256 + 256)
                t = opool.tile([128, 256], dt.float32, tag=tag, name=f"wp{k}_{half}")
                nc.tensor.matmul(t[:], at_s, xt_ch[k][:, sl], start=True, stop=True)
                nc.vector.tensor_copy(out=w_ch[k][:, sl], in_=t[:])

        # ---- prep: PE warmup + the single critical w half (i-cols 0:256) ----
        wt = opool.tile([128, 512], dt.float32, tag="oa0", name="warmp")
        for r in range(8):
            nc.tensor.matmul(
                wt[:, r * 64 : (r + 1) * 64],
                warm[:],
                warm[:, 0:64],
                start=True,
                stop=True,
            )
        emit_wprep(0, "oa0", half=0)

        # ---- main loop ----
        ngroups = (NJB + G - 1) // G
        # all remaining prep (v chunks, later w chunks) is injected into ic 0's
        # groups through the oa-tag psum banks, once its xT chunk has landed;
        # attn@v lags correspondingly but catches up (pt pool gives slack)
        prep_at = {
            0: (("v", 0, "oa0"), ("v", 1, "oa1")),
            1: (("v", 2, "oa0"), ("v", 3, "oa1")),
            2: (("v", 4, "oa0"), ("v", 5, "oa1")),
            3: (("v", 6, "oa0"), ("v", 7, "oa1")),
            4: (("wh", 0, "oa0"), ("w", 1, "oa1")),
            5: (("w", 2, "oa0"), ("w", 3, "oa1")),
        }

        def emit_energy(icn, jg, gsz):
            et = epool.tile([128, G, IC], dt.float32, tag="et", name=f"et{icn}_{jg}")
            wsl = w_ch[icn // 2][:, (icn % 2) * IC : (icn % 2 + 1) * IC]
            for g in range(gsz):
                jb = jg * G + g
                nc.tensor.matmul(
                    et[:, g, :],
                    xt_ch[jb // 4][:, (jb % 4) * 128 : (jb % 4 + 1) * 128],
                    wsl,
                    start=True,
                    stop=True,
                )
            return et

        def emit_exp(icn, jg, gsz):
            et = ets.pop((icn, jg))
            pt = ptpool.tile(
                [128, G, IC], dt.bfloat16, tag="pt", name=f"pt{icn}_{jg}"
            )
            nc.scalar.activation(
                out=pt[:, :gsz, :],
                in_=et[:, :gsz, :],
                func=mybir.ActivationFunctionType.Exp,
                bias=shiftb[:],
            )
            return pt

        def emit_attnv(icn, jg, gsz, pt):
            oa = oa_by_ic[icn]
            for g in range(gsz):
                jb = jg * G + g
                for it in range(NIT):
                    nc.tensor.matmul(
                        oa[it][:],
                        pt[:, g, it * 128 : (it + 1) * 128],
                        v_ch[jb // 4][:, jb % 4, 0:129],
                        start=(jb == 0),
                        stop=(jb == NJB - 1),
                    )

        def alloc_oa(icn):
            oa_by_ic[icn] = [
                opool.tile([128, 129], dt.float32, tag=f"oa{k}", name=f"oa{k}_{icn}")
                for k in range(NIT)
            ]

        def emit_blend(icn):
            oa = oa_by_ic[icn]
            for it in range(NIT):
                ti = icn * NIT + it
                # single fast PSUM read frees the accumulator bank quickly
                osb = osb_pool.tile([128, 129], dt.float32, tag="osb", name=f"osb{ti}")
                nc.vector.tensor_copy(out=osb[:], in_=oa[it][:])
                rs = spool.tile([128, 1], dt.float32, tag="rs", name=f"rs{ti}")
                nc.vector.reciprocal(rs[:], osb[:, 128:129])
                nc.vector.tensor_scalar(
                    out=rs[:],
                    in0=rs[:],
                    scalar1=gam[:],
                    scalar2=None,
                    op0=mybir.AluOpType.mult,
                )
                xr = xrpool.tile([128, 128], dt.float32, tag="xr", name=f"xr{ti}")
                nc.sync.dma_start(out=xr[:], in_=xres_d[ti])
                ot = outpool.tile([128, 128], dt.float32, tag="ot", name=f"ot{ti}")
                nc.vector.tensor_scalar(
                    out=ot[:],
                    in0=osb[:, 0:128],
                    scalar1=rs[:],
                    scalar2=None,
                    op0=mybir.AluOpType.mult,
                )
                nc.vector.tensor_tensor(
                    out=ot[:], in0=ot[:], in1=xr[:], op=mybir.AluOpType.add
                )
                nc.sync.dma_start(out=out_d[ti], in_=ot[:])

        ets = {}
        oa_by_ic = {}
        pts = {}

        # ---- i-chunk 0: energies/exps run at full cadence while all prep
        # (v chunks, remaining w) flows through the oa psum banks; the attn@v
        # matmuls for ic 0 are deferred until the oa banks are free ----
        ets[(0, 0)] = emit_energy(0, 0, G)
        for jg in range(ngroups):
            gsz = min(G, NJB - jg * G)
            if jg + 1 < ngroups:
                ets[(0, jg + 1)] = emit_energy(0, jg + 1, min(G, NJB - (jg + 1) * G))
            pts[jg] = emit_exp(0, jg, gsz)
            for kind, idx, tag in prep_at.get(jg, ()):
                if kind == "v":
                    emit_vprep(idx, tag)
                elif kind == "w":
                    emit_wprep(idx, tag)
                else:
                    emit_wprep(idx, tag, half=1)
        ets[(1, 0)] = emit_energy(1, 0, G)
        alloc_oa(0)
        for jg in range(ngroups):
            emit_attnv(0, jg, min(G, NJB - jg * G), pts.pop(jg))
            if jg == 0:
                ets[(1, 1)] = emit_energy(1, 1, G)
        emit_blend(0)

        # ---- i-chunks 1..: flat schedule with one-group PE lookahead ACROSS
        # chunk boundaries, so the scalar engine never waits on energy ----
        flat = [(icn, jg) for icn in range(1, NICH) for jg in range(ngroups)]
        for fk, (icn, jg) in enumerate(flat):
            gsz = min(G, NJB - jg * G)
            if fk + 1 < len(flat) and flat[fk + 1] not in ets:
                nicn, njg = flat[fk + 1]
                ets[flat[fk + 1]] = emit_energy(nicn, njg, min(G, NJB - njg * G))
            pt = emit_exp(icn, jg, gsz)
            if jg == 0:
                alloc_oa(icn)
            emit_attnv(icn, jg, gsz, pt)
            if jg == ngroups - 1:
                emit_blend(icn)

    nc.finalize()
    return nc


def get_nc():
    if "nc" not in _NC_CACHE:
        _NC_CACHE["nc"] = _build_nc()
    return _NC_CACHE["nc"]


def make_in_maps(x, Wq, Wk, Wv, gamma):
    x = np.asarray(x, dtype=np.float32)
    Wq = np.asarray(Wq, dtype=np.float32)
    Wk = np.asarray(Wk, dtype=np.float32)
    Wv = np.asarray(Wv, dtype=np.float32)
    gamma = np.asarray(gamma, dtype=np.float32)

    xf = x.reshape(B, N, C)
    A = Wq @ Wk.T
    aw = np.ascontiguousarray(
        np.concatenate([A.T.astype(np.float16), Wv.astype(np.float16)], axis=1)
    )  # [128, 256] = [A^T | Wv]
    gam = gamma.reshape(1, 1)

    in_maps = []
    for c in range(NCORES):
        b, ih = c // 2, c % 2
        xT = xf[b].T.astype(np.float16)  # [128, 4096]
        # rotate the j-order so this core's own i-rows are columns 0:NI
        # (softmax sums over j, so any j-order works as long as v matches)
        xTr = np.ascontiguousarray(np.roll(xT, -ih * NI, axis=1))
        sl = slice(ih * NI, (ih + 1) * NI)
        in_maps.append(
            {
                "xT": xTr,
                "xres": np.ascontiguousarray(
                    xf[b][sl].reshape(NI // 128, 128, 128)
                ),
                "aw": aw,
                "gam": gam,
            }
        )
    return in_maps


def assemble_out(results):
    outs = [np.asarray(results[c]["out"]).reshape(NI, C) for c in range(NCORES)]
    full = np.stack(
        [np.concatenate([outs[2 * b], outs[2 * b + 1]], axis=0) for b in range(B)]
    )
    return full.reshape(B, Dd, Hh, Ww, C).astype(np.float32)


def kernel(x, Wq, Wk, Wv, gamma):
    from concourse.bass_utils import run_bass_kernel_spmd

    nc = get_nc()
    in_maps = make_in_maps(x, Wq, Wk, Wv, gamma)
    res = run_bass_kernel_spmd(nc, in_maps, core_ids=list(range(NCORES)))
    return assemble_out(res.results)


# revision 33
# speedup vs baseline: 1.1976x; 1.0324x over previous
"""Position-attention kernel for Trainium2 (8 NeuronCores, SPMD data-parallel).

Math (per batch b):
    q = X Wq ; k = X Wk ; v = X Wv          (X = x[b] reshaped [N, C], N=4096, C=128)
    energy[i, j] = k_i . q_j
    attn = softmax(energy, axis=-1)
    out = gamma * (attn @ v) + X

Kernel restructuring:
    energy = X A X^T with A = Wq Wk^T, computed transposed as
    eT[j, i] = sum_c xT[c, j] * w[c, i]  where  w = A @ X_i^T  (tiny prep matmul).
    eT lands in PSUM with j on partitions and is exp'd (softmax is shift/scale
    invariant) by the scalar engine directly into SBUF as bf16 -> already in
    the right layout to be the stationary operand of the attn@v matmul (no
    transposes anywhere). A ones-column appended to v gives the softmax
    denominator for free.

Sharding: 8 cores = (4 batches) x (2 halves of the 4096 output rows).
"""

import numpy as np

B, Dd, Hh, Ww, C = 4, 16, 16, 16, 128
N = Dd * Hh * Ww            # 4096 sequence positions (j)
NCORES = 8
NI = (B * N) // NCORES      # 2048 output rows per core (i)
NJB = N // 128              # 32 j-blocks
G = 6                       # j-blocks per exp group (PSUM: 2*3 + 2 banks = 8)
IC = 256                    # i-chunk (2 accumulator tiles of 128 rows)
NICH = NI // IC             # 8 i-chunks
NIT = IC // 128             # 2 i-tiles per chunk
SHIFT = 32.0                # softmax shift (cancels exactly in normalization)

_NC_CACHE = {}


def _build_nc():
    from contextlib import ExitStack

    import concourse.bacc as bacc
    import concourse.bass as bass
    import concourse.mybir as mybir
    import concourse.tile as tile

    dt = mybir.dt
    nc = bacc.Bacc(target_bir_lowering=False)

    xT_d = nc.declare_dram_parameter("xT", [128, N], dt.float16, isOutput=False)
    xres_d = nc.declare_dram_parameter(
        "xres", [NI // 128, 128, 128], dt.float32, isOutput=False
    )
    # weights packed [A^T | Wv] along columns to halve DMA count
    aw_d = nc.declare_dram_parameter("aw", [128, 256], dt.float16, isOutput=False)
    gam_d = nc.declare_dram_parameter("gam", [1, 1], dt.float32, isOutput=False)
    out_d = nc.declare_dram_parameter(
        "out", [NI // 128, 128, 128], dt.float32, isOutput=True
    )

    NCH = N // 512   # 8 column chunks of xT
    NWC = NI // 512  # 4 column chunks of xTi / w
    with tile.TileContext(nc) as tc, ExitStack() as ctx:
        persist = ctx.enter_context(tc.tile_pool(name="persist", bufs=1))

        # warm up the exp table load while DMAs run
        dummy = persist.tile([1, 1], dt.float32)
        nc.vector.memset(dummy[:], 0.0)
        nc.scalar.activation(
            out=dummy[:], in_=dummy[:], func=mybir.ActivationFunctionType.Exp
        )
        # zeroed operand for PE-warmup matmuls
        warm = persist.tile([128, 128], dt.float16)
        nc.vector.memset(warm[:], 0.0)

        # DMA order mirrors the critical path: tiny queue-warmers first, then
        # aw -> xti[0:256] -> xt0/xt1 on the sync queue feed the first energy
        # group; the scalar queue carries the rest.
        qw = persist.tile([1, 4], dt.float16)
        nc.sync.dma_start(out=qw[0:1, 0:2], in_=aw_d[0:1, 0:2])
        nc.scalar.dma_start(out=qw[0:1, 2:4], in_=aw_d[0:1, 2:4])
        aw = persist.tile([128, 2, 128], dt.float16)
        xt_ch = [
            persist.tile([128, 512], dt.float16, name=f"xt{jc}") for jc in range(NCH)
        ]
        v_ch = [
            persist.tile([128, 4, 132], dt.bfloat16, name=f"v{jc}")
            for jc in range(NCH)
        ]

        def ld_xt(eng, jc):
            eng.dma_start(out=xt_ch[jc][:], in_=xT_d[:, jc * 512 : (jc + 1) * 512])

        # x is j-rotated on the host so each core's own i-rows are chunks 0-3;
        # interleave the two HW queues so the first-group inputs land first
        nc.sync.dma_start(out=aw[:], in_=aw_d[:, :])
        for jc, eng in [(0, nc.sync), (1, nc.scalar), (2, nc.scalar), (3, nc.sync),
                        (4, nc.scalar), (5, nc.sync), (6, nc.scalar), (7, nc.sync)]:
            ld_xt(eng, jc)
        gam = persist.tile([128, 1], dt.float32)
        gam_ap = gam_d[:, :]
        nc.gpsimd.dma_start(
            out=gam[:],
            in_=bass.AP(
                tensor=gam_ap.tensor, offset=gam_ap.offset, ap=[[0, 128], [1, 1]]
            ),
        )
        shiftb = persist.tile([128, 1], dt.float32)
        nc.vector.memset(shiftb[:], -SHIFT)
        for jc in range(NCH):
            nc.vector.memset(v_ch[jc][:, :, 128:129], 1.0)

        at_s = aw[:, 0, :]
        wv_s = aw[:, 1, :]
        w_ch = [persist.tile([128, 512], dt.float16, name=f"w{k}") for k in range(NWC)]

        epool = ctx.enter_context(tc.tile_pool(name="epsum", bufs=2, space="PSUM"))
        opool = ctx.enter_context(tc.tile_pool(name="opsum", bufs=1, space="PSUM"))
        ptpool = ctx.enter_context(tc.tile_pool(name="ptp", bufs=8))
        spool = ctx.enter_context(tc.tile_pool(name="small", bufs=8))
        osb_pool = ctx.enter_context(tc.tile_pool(name="osb", bufs=3))
        xrpool = ctx.enter_context(tc.tile_pool(name="xrp", bufs=3))
        outpool = ctx.enter_context(tc.tile_pool(name="outp", bufs=3))

        def emit_vprep(jc, tag):
            t = opool.tile([128, 4, 128], dt.float32, tag=tag, name=f"vp{jc}")
            for k in range(4):
                nc.tensor.matmul(
                    t[:, k, :],
                    xt_ch[jc][:, k * 128 : (k + 1) * 128],
                    wv_s,
                    start=True,
                    stop=True,
                )
            nc.vector.tensor_copy(out=v_ch[jc][:, :, 0:128], in_=t[:])

        def emit_wprep(k, tag, half=None):
            if half is None:
                t = opool.tile([128, 512], dt.float32, tag=tag, name=f"wp{k}")
                nc.tensor.matmul(t[:], at_s, xt_ch[k][:], start=True, stop=True)
                nc.vector.tensor_copy(out=w_ch[k][:], in_=t[:])
            else:
                sl = slice(half * 256, half * 256 + 256)
                t = opool.tile([128, 256], dt.float32, tag=tag, name=f"wp{k}_{half}")
                nc.tensor.matmul(t[:], at_s, xt_ch[k][:, sl], start=True, stop=True)
                nc.vector.tensor_copy(out=w_ch[k][:, sl], in_=t[:])

        # ---- prep: PE warmup + the single critical w half (i-cols 0:256) ----
        wt = opool.tile([128, 512], dt.float32, tag="oa0", name="warmp")
        for r in range(8):
            nc.tensor.matmul(
                wt[:, r * 64 : (r + 1) * 64],
                warm[:],
                warm[:, 0:64],
                start=True,
                stop=True,
            )
        emit_wprep(0, "oa0", half=0)

        # ---- main loop ----
        ngroups = (NJB + G - 1) // G
        # ic 0 starts with a chunk-0-only group so the first exp waits on the
        # minimum set of DMAs; later ics use the regular split
        GROUPS0 = [(0, 4), (4, 6), (10, 6), (16, 6), (22, 6), (28, 4)]
        GROUPSN = [(0, 6), (6, 6), (12, 6), (18, 6), (24, 6), (30, 2)]

        def groups_of(icn):
            return GROUPS0 if icn == 0 else GROUPSN

        # all remaining prep (v chunks, later w chunks) is injected into ic 0's
        # groups through the oa-tag psum banks, once its xT chunk has landed;
        # attn@v lags correspondingly but catches up (pt pool gives slack)
        prep_at = {
            0: (("v", 0, "oa0"),),
            1: (("v", 1, "oa1"), ("v", 2, "oa0")),
            2: (("v", 3, "oa1"), ("v", 4, "oa0")),
            3: (("v", 5, "oa1"), ("v", 6, "oa0")),
            4: (("v", 7, "oa1"), ("wh", 0, "oa0")),
            5: (("w", 1, "oa1"), ("w", 2, "oa0"), ("w", 3, "oa1")),
        }

        def emit_energy(icn, gi):
            jb0, gsz = groups_of(icn)[gi]
            et = epool.tile([128, G, IC], dt.float32, tag="et", name=f"et{icn}_{gi}")
            wsl = w_ch[icn // 2][:, (icn % 2) * IC : (icn % 2 + 1) * IC]
            for g in range(gsz):
                jb = jb0 + g
                nc.tensor.matmul(
                    et[:, g, :],
                    xt_ch[jb // 4][:, (jb % 4) * 128 : (jb % 4 + 1) * 128],
                    wsl,
                    start=True,
                    stop=True,
                )
            return et

        def emit_exp(icn, gi):
            gsz = groups_of(icn)[gi][1]
            et = ets.pop((icn, gi))
            pt = ptpool.tile(
                [128, G, IC], dt.bfloat16, tag="pt", name=f"pt{icn}_{gi}"
            )
            nc.scalar.activation(
                out=pt[:, :gsz, :],
                in_=et[:, :gsz, :],
                func=mybir.ActivationFunctionType.Exp,
                bias=shiftb[:],
            )
            return pt

        def emit_attnv(icn, gi, pt):
            jb0, gsz = groups_of(icn)[gi]
            oa = oa_by_ic[icn]
            for g in range(gsz):
                jb = jb0 + g
                for it in range(NIT):
                    nc.tensor.matmul(
                        oa[it][:],
                        pt[:, g, it * 128 : (it + 1) * 128],
                        v_ch[jb // 4][:, jb % 4, 0:129],
                        start=(jb == 0),
                        stop=(jb == NJB - 1),
                    )

        def alloc_oa(icn):
            oa_by_ic[icn] = [
                opool.tile([128, 129], dt.float32, tag=f"oa{k}", name=f"oa{k}_{icn}")
                for k in range(NIT)
            ]

        def emit_blend(icn):
            oa = oa_by_ic[icn]
            for it in range(NIT):
                ti = icn * NIT + it
                # single fast PSUM read frees the accumulator bank quickly
                osb = osb_pool.tile([128, 129], dt.float32, tag="osb", name=f"osb{ti}")
                nc.vector.tensor_copy(out=osb[:], in_=oa[it][:])
                rs = spool.tile([128, 1], dt.float32, tag="rs", name=f"rs{ti}")
                nc.vector.reciprocal(rs[:], osb[:, 128:129])
                nc.vector.tensor_scalar(
                    out=rs[:],
                    in0=rs[:],
                    scalar1=gam[:],
                    scalar2=None,
                    op0=mybir.AluOpType.mult,
                )
                xr = xrpool.tile([128, 128], dt.float32, tag="xr", name=f"xr{ti}")
                nc.sync.dma_start(out=xr[:], in_=xres_d[ti])
                ot = outpool.tile([128, 128], dt.float32, tag="ot", name=f"ot{ti}")
                nc.vector.tensor_scalar(
                    out=ot[:],
                    in0=osb[:, 0:128],
                    scalar1=rs[:],
                    scalar2=None,
                    op0=mybir.AluOpType.mult,
                )
                nc.vector.tensor_tensor(
                    out=ot[:], in0=ot[:], in1=xr[:], op=mybir.AluOpType.add
                )
                nc.sync.dma_start(out=out_d[ti], in_=ot[:])

        ets = {}
        oa_by_ic = {}
        pts = {}

        # ---- i-chunk 0: energies/exps run at full cadence while all prep
        # (v chunks, remaining w) flows through the oa psum banks; the attn@v
        # matmuls for ic 0 are deferred until the oa banks are free ----
        ets[(0, 0)] = emit_energy(0, 0)
        for gi in range(ngroups):
            if gi + 1 < ngroups:
                ets[(0, gi + 1)] = emit_energy(0, gi + 1)
            pts[gi] = emit_exp(0, gi)
            for kind, idx, tag in prep_at.get(gi, ()):
                if kind == "v":
                    emit_vprep(idx, tag)
                elif kind == "w":
                    emit_wprep(idx, tag)
                else:
                    emit_wprep(idx, tag, half=1)
        ets[(1, 0)] = emit_energy(1, 0)
        alloc_oa(0)
        for gi in range(ngroups):
            emit_attnv(0, gi, pts.pop(gi))
            if gi + 1 < ngroups:
                ets[(1, gi + 1)] = emit_energy(1, gi + 1)
        emit_blend(0)

        # ---- i-chunks 1..: flat schedule with one-group PE lookahead ACROSS
        # chunk boundaries, so the scalar engine never waits on energy ----
        flat = [(icn, gi) for icn in range(1, NICH) for gi in range(ngroups)]
        for fk, (icn, gi) in enumerate(flat):
            if fk + 1 < len(flat) and flat[fk + 1] not in ets:
                nicn, ngi = flat[fk + 1]
                ets[flat[fk + 1]] = emit_energy(nicn, ngi)
            pt = emit_exp(icn, gi)
            if gi == 0:
                alloc_oa(icn)
            emit_attnv(icn, gi, pt)
            if gi == ngroups - 1:
                emit_blend(icn)

    nc.finalize()
    return nc


def get_nc():
    if "nc" not in _NC_CACHE:
        _NC_CACHE["nc"] = _build_nc()
    return _NC_CACHE["nc"]


def make_in_maps(x, Wq, Wk, Wv, gamma):
    x = np.asarray(x, dtype=np.float32)
    Wq = np.asarray(Wq, dtype=np.float32)
    Wk = np.asarray(Wk, dtype=np.float32)
    Wv = np.asarray(Wv, dtype=np.float32)
    gamma = np.asarray(gamma, dtype=np.float32)

    xf = x.reshape(B, N, C)
    A = Wq @ Wk.T
    aw = np.ascontiguousarray(
        np.concatenate([A.T.astype(np.float16), Wv.astype(np.float16)], axis=1)
    )  # [128, 256] = [A^T | Wv]
    gam = gamma.reshape(1, 1)

    in_maps = []
    for c in range(NCORES):
        b, ih = c // 2, c % 2
        xT = xf[b].T.astype(np.float16)  # [128, 4096]
        # rotate the j-order so this core's own i-rows are columns 0:NI
        # (softmax sums over j, so any j-order works as long as v matches)
        xTr = np.ascontiguousarray(np.roll(xT, -ih * NI, axis=1))
        sl = slice(ih * NI, (ih + 1) * NI)
        in_maps.append(
            {
                "xT": xTr,
                "xres": np.ascontiguousarray(
                    xf[b][sl].reshape(NI // 128, 128, 128)
                ),
                "aw": aw,
                "gam": gam,
            }
        )
    return in_maps


def assemble_out(results):
    outs = [np.asarray(results[c]["out"]).reshape(NI, C) for c in range(NCORES)]
    full = np.stack(
        [np.concatenate([outs[2 * b], outs[2 * b + 1]], axis=0) for b in range(B)]
    )
    return full.reshape(B, Dd, Hh, Ww, C).astype(np.float32)


def kernel(x, Wq, Wk, Wv, gamma):
    from concourse.bass_utils import run_bass_kernel_spmd

    nc = get_nc()
    in_maps = make_in_maps(x, Wq, Wk, Wv, gamma)
    res = run_bass_kernel_spmd(nc, in_maps, core_ids=list(range(NCORES)))
    return assemble_out(res.results)


# revision 34
# speedup vs baseline: 1.2985x; 1.0843x over previous
"""Position-attention kernel for Trainium2 (8 NeuronCores, SPMD data-parallel).

Math (per batch b):
    q = X Wq ; k = X Wk ; v = X Wv          (X = x[b] reshaped [N, C], N=4096, C=128)
    energy[i, j] = k_i . q_j
    attn = softmax(energy, axis=-1)
    out = gamma * (attn @ v) + X

Kernel restructuring:
    energy = X A X^T with A = Wq Wk^T, computed transposed as
    eT[j, i] = sum_c xT[c, j] * w[c, i]  where  w = A @ X_i^T  (tiny prep matmul).
    eT lands in PSUM with j on partitions and is exp'd (softmax is shift/scale
    invariant) by the scalar engine directly into SBUF as bf16 -> already in
    the right layout to be the stationary operand of the attn@v matmul (no
    transposes anywhere). A ones-column appended to v gives the softmax
    denominator for free.

Sharding: 8 cores = (4 batches) x (2 halves of the 4096 output rows).
"""

import numpy as np

B, Dd, Hh, Ww, C = 4, 16, 16, 16, 128
N = Dd * Hh * Ww            # 4096 sequence positions (j)
NCORES = 8
NI = (B * N) // NCORES      # 2048 output rows per core (i)
NJB = N // 128              # 32 j-blocks
G = 6                       # j-blocks per exp group (PSUM: 2*3 + 2 banks = 8)
IC = 256                    # i-chunk (2 accumulator tiles of 128 rows)
NICH = NI // IC             # 8 i-chunks
NIT = IC // 128             # 2 i-tiles per chunk
SHIFT = 32.0                # softmax shift (cancels exactly in normalization)

_NC_CACHE = {}


def _build_nc():
    from contextlib import ExitStack

    import concourse.bacc as bacc
    import concourse.bass as bass
    import concourse.mybir as mybir
    import concourse.tile as tile

    dt = mybir.dt
    nc = bacc.Bacc(target_bir_lowering=False)

    xT_d = nc.declare_dram_parameter("xT", [128, N], dt.float16, isOutput=False)
    xres_d = nc.declare_dram_parameter(
        "xres", [NI // 128, 128, 128], dt.float32, isOutput=False
    )
    # weights packed [A^T | Wv] along columns to halve DMA count
    aw_d = nc.declare_dram_parameter("aw", [128, 256], dt.float16, isOutput=False)
    gam_d = nc.declare_dram_parameter("gam", [1, 1], dt.float32, isOutput=False)
    out_d = nc.declare_dram_parameter(
        "out", [NI // 128, 128, 128], dt.float32, isOutput=True
    )

    NCH = N // 512   # 8 column chunks of xT
    NWC = NI // 512  # 4 column chunks of xTi / w
    with tile.TileContext(nc) as tc, ExitStack() as ctx:
        persist = ctx.enter_context(tc.tile_pool(name="persist", bufs=1))

        # warm up the exp table load while DMAs run
        dummy = persist.tile([1, 1], dt.float32)
        nc.vector.memset(dummy[:], 0.0)
        nc.scalar.activation(
            out=dummy[:], in_=dummy[:], func=mybir.ActivationFunctionType.Exp
        )
        # zeroed operand for PE-warmup matmuls
        warm = persist.tile([128, 128], dt.float16)
        nc.vector.memset(warm[:], 0.0)

        # DMA order mirrors the critical path: tiny queue-warmers first, then
        # aw -> xti[0:256] -> xt0/xt1 on the sync queue feed the first energy
        # group; the scalar queue carries the rest.
        qw = persist.tile([1, 4], dt.float16)
        nc.sync.dma_start(out=qw[0:1, 0:2], in_=aw_d[0:1, 0:2])
        nc.scalar.dma_start(out=qw[0:1, 2:4], in_=aw_d[0:1, 2:4])
        aw = persist.tile([128, 2, 128], dt.float16)
        xt_ch = [
            persist.tile([128, 512], dt.float16, name=f"xt{jc}") for jc in range(NCH)
        ]
        v_ch = [
            persist.tile([128, 4, 132], dt.bfloat16, name=f"v{jc}")
            for jc in range(NCH)
        ]

        def ld_xt(eng, jc):
            eng.dma_start(out=xt_ch[jc][:], in_=xT_d[:, jc * 512 : (jc + 1) * 512])

        # x is j-rotated on the host so each core's own i-rows are chunks 0-3;
        # interleave the two HW queues so the first-group inputs land first
        nc.sync.dma_start(out=aw[:], in_=aw_d[:, :])
        for jc, eng in [(0, nc.sync), (1, nc.scalar), (2, nc.scalar), (3, nc.sync),
                        (4, nc.scalar), (5, nc.sync), (6, nc.scalar), (7, nc.sync)]:
            ld_xt(eng, jc)
        gam = persist.tile([128, 1], dt.float32)
        gam_ap = gam_d[:, :]
        nc.gpsimd.dma_start(
            out=gam[:],
            in_=bass.AP(
                tensor=gam_ap.tensor, offset=gam_ap.offset, ap=[[0, 128], [1, 1]]
            ),
        )
        shiftb = persist.tile([128, 1], dt.float32)
        nc.vector.memset(shiftb[:], -SHIFT)
        for jc in range(NCH):
            nc.vector.memset(v_ch[jc][:, :, 128:129], 1.0)

        at_s = aw[:, 0, :]
        wv_s = aw[:, 1, :]
        w_ch = [persist.tile([128, 512], dt.float16, name=f"w{k}") for k in range(NWC)]

        epool = ctx.enter_context(tc.tile_pool(name="epsum", bufs=2, space="PSUM"))
        opool = ctx.enter_context(tc.tile_pool(name="opsum", bufs=1, space="PSUM"))
        ptpool = ctx.enter_context(tc.tile_pool(name="ptp", bufs=8))
        spool = ctx.enter_context(tc.tile_pool(name="small", bufs=8))
        osb_pool = ctx.enter_context(tc.tile_pool(name="osb", bufs=3))
        xrpool = ctx.enter_context(tc.tile_pool(name="xrp", bufs=3))
        outpool = ctx.enter_context(tc.tile_pool(name="outp", bufs=3))

        def emit_vprep(jc, tag):
            t = opool.tile([128, 4, 128], dt.float32, tag=tag, name=f"vp{jc}")
            for k in range(4):
                nc.tensor.matmul(
                    t[:, k, :],
                    xt_ch[jc][:, k * 128 : (k + 1) * 128],
                    wv_s,
                    start=True,
                    stop=True,
                )
            nc.vector.tensor_copy(out=v_ch[jc][:, :, 0:128], in_=t[:])

        def emit_wprep(k, tag, half=None):
            if half is None:
                t = opool.tile([128, 512], dt.float32, tag=tag, name=f"wp{k}")
                nc.tensor.matmul(t[:], at_s, xt_ch[k][:], start=True, stop=True)
                nc.vector.tensor_copy(out=w_ch[k][:], in_=t[:])
            else:
                sl = slice(half * 256, half * 256 + 256)
                t = opool.tile([128, 256], dt.float32, tag=tag, name=f"wp{k}_{half}")
                nc.tensor.matmul(t[:], at_s, xt_ch[k][:, sl], start=True, stop=True)
                nc.vector.tensor_copy(out=w_ch[k][:, sl], in_=t[:])

        # ---- prep: PE warmup + the single critical w half (i-cols 0:256) ----
        wt = opool.tile([128, 512], dt.float32, tag="oa0", name="warmp")
        for r in range(8):
            nc.tensor.matmul(
                wt[:, r * 64 : (r + 1) * 64],
                warm[:],
                warm[:, 0:64],
                start=True,
                stop=True,
            )
        emit_wprep(0, "oa0", half=0)

        # ---- main loop ----
        ngroups = (NJB + G - 1) // G
        # ic 0 starts with a chunk-0-only group so the first exp waits on the
        # minimum set of DMAs; later ics use the regular split
        GROUPS0 = [(0, 4), (4, 6), (10, 6), (16, 6), (22, 6), (28, 4)]
        GROUPSN = [(0, 6), (6, 6), (12, 6), (18, 6), (24, 6), (30, 2)]

        def groups_of(icn):
            return GROUPS0 if icn == 0 else GROUPSN

        # all remaining prep (v chunks, later w chunks) is injected into ic 0's
        # groups through the oa-tag psum banks, once its xT chunk has landed;
        # attn@v lags correspondingly but catches up (pt pool gives slack)
        prep_at = {
            0: (("v", 0, "oa0"),),
            1: (("v", 1, "oa1"), ("v", 2, "oa0")),
            2: (("v", 3, "oa1"), ("v", 4, "oa0")),
            3: (("v", 5, "oa1"), ("v", 6, "oa0")),
            4: (("v", 7, "oa1"), ("wh", 0, "oa0")),
            5: (("w", 1, "oa1"), ("w", 2, "oa0"), ("w", 3, "oa1")),
        }

        def emit_energy(icn, gi):
            jb0, gsz = groups_of(icn)[gi]
            et = epool.tile([128, G, IC], dt.float32, tag="et", name=f"et{icn}_{gi}")
            wsl = w_ch[icn // 2][:, (icn % 2) * IC : (icn % 2 + 1) * IC]
            for g in range(gsz):
                jb = jb0 + g
                nc.tensor.matmul(
                    et[:, g, :],
                    xt_ch[jb // 4][:, (jb % 4) * 128 : (jb % 4 + 1) * 128],
                    wsl,
                    start=True,
                    stop=True,
                )
            return et

        def emit_exp(icn, gi):
            gsz = groups_of(icn)[gi][1]
            et = ets.pop((icn, gi))
            pt = ptpool.tile(
                [128, G, IC], dt.bfloat16, tag="pt", name=f"pt{icn}_{gi}"
            )
            nc.scalar.activation(
                out=pt[:, :gsz, :],
                in_=et[:, :gsz, :],
                func=mybir.ActivationFunctionType.Exp,
                bias=shiftb[:],
            )
            return pt

        def emit_attnv(icn, gi, pt):
            jb0, gsz = groups_of(icn)[gi]
            oa = oa_by_ic[icn]
            for g in range(gsz):
                jb = jb0 + g
                for it in range(NIT):
                    nc.tensor.matmul(
                        oa[it][:],
                        pt[:, g, it * 128 : (it + 1) * 128],
                        v_ch[jb // 4][:, jb % 4, 0:129],
                        start=(jb == 0),
                        stop=(jb == NJB - 1),
                    )

        def alloc_oa(icn):
            oa_by_ic[icn] = [
                opool.tile([128, 129], dt.float32, tag=f"oa{k}", name=f"oa{k}_{icn}")
                for k in range(NIT)
            ]

        def emit_blend(icn):
            oa = oa_by_ic[icn]
            for it in range(NIT):
                ti = icn * NIT + it
                # single fast PSUM read frees the accumulator bank quickly
                osb = osb_pool.tile([128, 129], dt.float32, tag="osb", name=f"osb{ti}")
                nc.vector.tensor_copy(out=osb[:], in_=oa[it][:])
                rs = spool.tile([128, 1], dt.float32, tag="rs", name=f"rs{ti}")
                nc.vector.reciprocal(rs[:], osb[:, 128:129])
                nc.vector.tensor_scalar(
                    out=rs[:],
                    in0=rs[:],
                    scalar1=gam[:],
                    scalar2=None,
                    op0=mybir.AluOpType.mult,
                )
                xr = xrpool.tile([128, 128], dt.float32, tag="xr", name=f"xr{ti}")
                nc.sync.dma_start(out=xr[:], in_=xres_d[ti])
                ot = outpool.tile([128, 128], dt.float32, tag="ot", name=f"ot{ti}")
                nc.vector.tensor_scalar(
                    out=ot[:],
                    in0=osb[:, 0:128],
                    scalar1=rs[:],
                    scalar2=None,
                    op0=mybir.AluOpType.mult,
                )
                nc.vector.tensor_tensor(
                    out=ot[:], in0=ot[:], in1=xr[:], op=mybir.AluOpType.add
                )
                nc.sync.dma_start(out=out_d[ti], in_=ot[:])

        ets = {}
        oa_by_ic = {}
        pts = {}

        # ---- i-chunk 0: energies/exps run at full cadence while all prep
        # (v chunks, remaining w) flows through the oa psum banks; the attn@v
        # matmuls for ic 0 are deferred until the oa banks are free ----
        ets[(0, 0)] = emit_energy(0, 0)
        for gi in range(ngroups):
            if gi + 1 < ngroups:
                ets[(0, gi + 1)] = emit_energy(0, gi + 1)
            pts[gi] = emit_exp(0, gi)
            for kind, idx, tag in prep_at.get(gi, ()):
                if kind == "v":
                    emit_vprep(idx, tag)
                elif kind == "w":
                    emit_wprep(idx, tag)
                else:
                    emit_wprep(idx, tag, half=1)
        ets[(1, 0)] = emit_energy(1, 0)
        alloc_oa(0)
        for gi in range(ngroups):
            emit_attnv(0, gi, pts.pop(gi))
            # keep feeding the energy pipeline through the deferred burst
            nxt = (1, gi + 1) if gi + 1 < ngroups else (2, 0)
            ets[nxt] = emit_energy(*nxt)
        emit_blend(0)

        # ---- i-chunks 1..: flat schedule with two-group PE lookahead ACROSS
        # chunk boundaries, so the scalar engine never waits on energy ----
        flat = [(icn, gi) for icn in range(1, NICH) for gi in range(ngroups)]
        for fk, (icn, gi) in enumerate(flat):
            for ahead in (1, 2):
                if fk + ahead < len(flat) and flat[fk + ahead] not in ets:
                    nicn, ngi = flat[fk + ahead]
                    ets[flat[fk + ahead]] = emit_energy(nicn, ngi)
            pt = emit_exp(icn, gi)
            if gi == 0:
                alloc_oa(icn)
            emit_attnv(icn, gi, pt)
            if gi == ngroups - 1:
                emit_blend(icn)

    nc.finalize()
    return nc


def get_nc():
    if "nc" not in _NC_CACHE:
        _NC_CACHE["nc"] = _build_nc()
    return _NC_CACHE["nc"]


def make_in_maps(x, Wq, Wk, Wv, gamma):
    x = np.asarray(x, dtype=np.float32)
    Wq = np.asarray(Wq, dtype=np.float32)
    Wk = np.asarray(Wk, dtype=np.float32)
    Wv = np.asarray(Wv, dtype=np.float32)
    gamma = np.asarray(gamma, dtype=np.float32)

    xf = x.reshape(B, N, C)
    A = Wq @ Wk.T
    aw = np.ascontiguousarray(
        np.concatenate([A.T.astype(np.float16), Wv.astype(np.float16)], axis=1)
    )  # [128, 256] = [A^T | Wv]
    gam = gamma.reshape(1, 1)

    in_maps = []
    for c in range(NCORES):
        b, ih = c // 2, c % 2
        xT = xf[b].T.astype(np.float16)  # [128, 4096]
        # rotate the j-order so this core's own i-rows are columns 0:NI
        # (softmax sums over j, so any j-order works as long as v matches)
        xTr = np.ascontiguousarray(np.roll(xT, -ih * NI, axis=1))
        sl = slice(ih * NI, (ih + 1) * NI)
        in_maps.append(
            {
                "xT": xTr,
                "xres": np.ascontiguousarray(
                    xf[b][sl].reshape(NI // 128, 128, 128)
                ),
                "aw": aw,
                "gam": gam,
            }
        )
    return in_maps


def assemble_out(results):
    outs = [np.asarray(results[c]["out"]).reshape(NI, C) for c in range(NCORES)]
    full = np.stack(
        [np.concatenate([outs[2 * b], outs[2 * b + 1]], axis=0) for b in range(B)]
    )
    return full.reshape(B, Dd, Hh, Ww, C).astype(np.float32)


def kernel(x, Wq, Wk, Wv, gamma):
    from concourse.bass_utils import run_bass_kernel_spmd

    nc = get_nc()
    in_maps = make_in_maps(x, Wq, Wk, Wv, gamma)
    res = run_bass_kernel_spmd(nc, in_maps, core_ids=list(range(NCORES)))
    return assemble_out(res.results)


# revision 36
# speedup vs baseline: 1.3125x; 1.0108x over previous
"""Position-attention kernel for Trainium2 (8 NeuronCores, SPMD data-parallel).

Math (per batch b):
    q = X Wq ; k = X Wk ; v = X Wv          (X = x[b] reshaped [N, C], N=4096, C=128)
    energy[i, j] = k_i . q_j
    attn = softmax(energy, axis=-1)
    out = gamma * (attn @ v) + X

Kernel restructuring:
    energy = X A X^T with A = Wq Wk^T, computed transposed as
    eT[j, i] = sum_c xT[c, j] * w[c, i]  where  w = A @ X_i^T  (tiny prep matmul).
    eT lands in PSUM with j on partitions and is exp'd (softmax is shift/scale
    invariant) by the scalar engine directly into SBUF as bf16 -> already in
    the right layout to be the stationary operand of the attn@v matmul (no
    transposes anywhere). A ones-column appended to v gives the softmax
    denominator for free.

Sharding: 8 cores = (4 batches) x (2 halves of the 4096 output rows).
"""

import numpy as np

B, Dd, Hh, Ww, C = 4, 16, 16, 16, 128
N = Dd * Hh * Ww            # 4096 sequence positions (j)
NCORES = 8
NI = (B * N) // NCORES      # 2048 output rows per core (i)
NJB = N // 128              # 32 j-blocks
G = 6                       # j-blocks per exp group (PSUM: 2*3 + 2 banks = 8)
IC = 256                    # i-chunk (2 accumulator tiles of 128 rows)
NICH = NI // IC             # 8 i-chunks
NIT = IC // 128             # 2 i-tiles per chunk
SHIFT = 32.0                # softmax shift (cancels exactly in normalization)

_NC_CACHE = {}


def _build_nc():
    from contextlib import ExitStack

    import concourse.bacc as bacc
    import concourse.bass as bass
    import concourse.mybir as mybir
    import concourse.tile as tile

    dt = mybir.dt
    nc = bacc.Bacc(target_bir_lowering=False)

    xT_d = nc.declare_dram_parameter("xT", [128, N], dt.float16, isOutput=False)
    xres_d = nc.declare_dram_parameter(
        "xres", [NI // 128, 128, 128], dt.float32, isOutput=False
    )
    # weights packed [A^T | Wv] along columns to halve DMA count
    aw_d = nc.declare_dram_parameter("aw", [128, 256], dt.float16, isOutput=False)
    gam_d = nc.declare_dram_parameter("gam", [1, 1], dt.float32, isOutput=False)
    out_d = nc.declare_dram_parameter(
        "out", [NI // 128, 128, 128], dt.float32, isOutput=True
    )

    NCH = N // 512   # 8 column chunks of xT
    NWC = NI // 512  # 4 column chunks of xTi / w
    with tile.TileContext(nc) as tc, ExitStack() as ctx:
        persist = ctx.enter_context(tc.tile_pool(name="persist", bufs=1))

        # warm up the exp table load while DMAs run
        dummy = persist.tile([1, 1], dt.float32)
        nc.vector.memset(dummy[:], 0.0)
        nc.scalar.activation(
            out=dummy[:], in_=dummy[:], func=mybir.ActivationFunctionType.Exp
        )
        # zeroed operand for PE-warmup matmuls
        warm = persist.tile([128, 128], dt.float16)
        nc.vector.memset(warm[:], 0.0)

        # DMA order mirrors the critical path: tiny queue-warmers first, then
        # aw -> xti[0:256] -> xt0/xt1 on the sync queue feed the first energy
        # group; the scalar queue carries the rest.
        qw = persist.tile([1, 4], dt.float16)
        nc.sync.dma_start(out=qw[0:1, 0:2], in_=aw_d[0:1, 0:2])
        nc.scalar.dma_start(out=qw[0:1, 2:4], in_=aw_d[0:1, 2:4])
        aw = persist.tile([128, 2, 128], dt.float16)
        xt_ch = [
            persist.tile([128, 512], dt.float16, name=f"xt{jc}") for jc in range(NCH)
        ]
        v_ch = [
            persist.tile([128, 4, 132], dt.bfloat16, name=f"v{jc}")
            for jc in range(NCH)
        ]

        def ld_xt(eng, jc):
            eng.dma_start(out=xt_ch[jc][:], in_=xT_d[:, jc * 512 : (jc + 1) * 512])

        # x is j-rotated on the host so each core's own i-rows are chunks 0-3;
        # interleave the two HW queues so the first-group inputs land first
        nc.sync.dma_start(out=aw[:], in_=aw_d[:, :])
        nc.scalar.dma_start(out=xt_ch[0][:, 0:256], in_=xT_d[:, 0:256])
        nc.sync.dma_start(out=xt_ch[0][:, 256:512], in_=xT_d[:, 256:512])
        for jc, eng in [(1, nc.scalar), (2, nc.scalar), (3, nc.sync),
                        (4, nc.scalar), (5, nc.sync), (6, nc.scalar), (7, nc.sync)]:
            ld_xt(eng, jc)
        gam = persist.tile([128, 1], dt.float32)
        gam_ap = gam_d[:, :]
        nc.gpsimd.dma_start(
            out=gam[:],
            in_=bass.AP(
                tensor=gam_ap.tensor, offset=gam_ap.offset, ap=[[0, 128], [1, 1]]
            ),
        )
        shiftb = persist.tile([128, 1], dt.float32)
        nc.vector.memset(shiftb[:], -SHIFT)
        for jc in range(NCH):
            nc.vector.memset(v_ch[jc][:, :, 128:129], 1.0)

        at_s = aw[:, 0, :]
        wv_s = aw[:, 1, :]
        w_ch = [persist.tile([128, 512], dt.float16, name=f"w{k}") for k in range(NWC)]

        epool = ctx.enter_context(tc.tile_pool(name="epsum", bufs=2, space="PSUM"))
        opool = ctx.enter_context(tc.tile_pool(name="opsum", bufs=1, space="PSUM"))
        ptpool = ctx.enter_context(tc.tile_pool(name="ptp", bufs=8))
        spool = ctx.enter_context(tc.tile_pool(name="small", bufs=8))
        osb_pool = ctx.enter_context(tc.tile_pool(name="osb", bufs=3))
        xrpool = ctx.enter_context(tc.tile_pool(name="xrp", bufs=3))
        outpool = ctx.enter_context(tc.tile_pool(name="outp", bufs=3))

        def emit_vprep(jc, tag):
            t = opool.tile([128, 4, 128], dt.float32, tag=tag, name=f"vp{jc}")
            for k in range(4):
                nc.tensor.matmul(
                    t[:, k, :],
                    xt_ch[jc][:, k * 128 : (k + 1) * 128],
                    wv_s,
                    start=True,
                    stop=True,
                )
            nc.vector.tensor_copy(out=v_ch[jc][:, :, 0:128], in_=t[:])

        def emit_wprep(k, tag, half=None):
            if half is None:
                t = opool.tile([128, 512], dt.float32, tag=tag, name=f"wp{k}")
                nc.tensor.matmul(t[:], at_s, xt_ch[k][:], start=True, stop=True)
                nc.vector.tensor_copy(out=w_ch[k][:], in_=t[:])
            else:
                sl = slice(half * 256, half * 256 + 256)
                t = opool.tile([128, 256], dt.float32, tag=tag, name=f"wp{k}_{half}")
                nc.tensor.matmul(t[:], at_s, xt_ch[k][:, sl], start=True, stop=True)
                nc.vector.tensor_copy(out=w_ch[k][:, sl], in_=t[:])

        # ---- prep: PE warmup + the single critical w half (i-cols 0:256) ----
        wt = opool.tile([128, 512], dt.float32, tag="oa0", name="warmp")
        for r in range(8):
            nc.tensor.matmul(
                wt[:, r * 64 : (r + 1) * 64],
                warm[:],
                warm[:, 0:64],
                start=True,
                stop=True,
            )
        emit_wprep(0, "oa0", half=0)

        # ---- main loop ----
        ngroups = (NJB + G - 1) // G
        # ic 0 starts with a chunk-0-only group so the first exp waits on the
        # minimum set of DMAs; later ics use the regular split
        GROUPS0 = [(0, 4), (4, 6), (10, 6), (16, 6), (22, 6), (28, 4)]
        GROUPSN = [(0, 6), (6, 6), (12, 6), (18, 6), (24, 6), (30, 2)]

        def groups_of(icn):
            return GROUPS0 if icn == 0 else GROUPSN

        # all remaining prep (v chunks, later w chunks) is injected into ic 0's
        # groups through the oa-tag psum banks, once its xT chunk has landed;
        # attn@v lags correspondingly but catches up (pt pool gives slack)
        prep_at = {
            0: (("v", 0, "oa0"),),
            1: (("v", 1, "oa1"), ("v", 2, "oa0")),
            2: (("v", 3, "oa1"), ("v", 4, "oa0")),
            3: (("v", 5, "oa1"), ("v", 6, "oa0")),
            4: (("v", 7, "oa1"), ("wh", 0, "oa0")),
            5: (("w", 1, "oa1"), ("w", 2, "oa0"), ("w", 3, "oa1")),
        }

        def emit_energy(icn, gi):
            jb0, gsz = groups_of(icn)[gi]
            et = epool.tile([128, G, IC], dt.float32, tag="et", name=f"et{icn}_{gi}")
            wsl = w_ch[icn // 2][:, (icn % 2) * IC : (icn % 2 + 1) * IC]
            for g in range(gsz):
                jb = jb0 + g
                nc.tensor.matmul(
                    et[:, g, :],
                    xt_ch[jb // 4][:, (jb % 4) * 128 : (jb % 4 + 1) * 128],
                    wsl,
                    start=True,
                    stop=True,
                )
            return et

        def emit_exp(icn, gi):
            gsz = groups_of(icn)[gi][1]
            et = ets.pop((icn, gi))
            pt = ptpool.tile(
                [128, G, IC], dt.bfloat16, tag="pt", name=f"pt{icn}_{gi}"
            )
            nc.scalar.activation(
                out=pt[:, :gsz, :],
                in_=et[:, :gsz, :],
                func=mybir.ActivationFunctionType.Exp,
                bias=shiftb[:],
            )
            return pt

        def emit_attnv(icn, gi, pt):
            jb0, gsz = groups_of(icn)[gi]
            oa = oa_by_ic[icn]
            for g in range(gsz):
                jb = jb0 + g
                for it in range(NIT):
                    nc.tensor.matmul(
                        oa[it][:],
                        pt[:, g, it * 128 : (it + 1) * 128],
                        v_ch[jb // 4][:, jb % 4, 0:129],
                        start=(jb == 0),
                        stop=(jb == NJB - 1),
                    )

        def alloc_oa(icn):
            oa_by_ic[icn] = [
                opool.tile([128, 129], dt.float32, tag=f"oa{k}", name=f"oa{k}_{icn}")
                for k in range(NIT)
            ]

        def emit_blend(icn):
            oa = oa_by_ic[icn]
            for it in range(NIT):
                ti = icn * NIT + it
                # single fast PSUM read frees the accumulator bank quickly
                osb = osb_pool.tile([128, 129], dt.float32, tag="osb", name=f"osb{ti}")
                nc.vector.tensor_copy(out=osb[:], in_=oa[it][:])
                rs = spool.tile([128, 1], dt.float32, tag="rs", name=f"rs{ti}")
                nc.vector.reciprocal(rs[:], osb[:, 128:129])
                nc.vector.tensor_scalar(
                    out=rs[:],
                    in0=rs[:],
                    scalar1=gam[:],
                    scalar2=None,
                    op0=mybir.AluOpType.mult,
                )
                xr = xrpool.tile([128, 128], dt.float32, tag="xr", name=f"xr{ti}")
                nc.sync.dma_start(out=xr[:], in_=xres_d[ti])
                ot = outpool.tile([128, 128], dt.float32, tag="ot", name=f"ot{ti}")
                nc.vector.tensor_scalar(
                    out=ot[:],
                    in0=osb[:, 0:128],
                    scalar1=rs[:],
                    scalar2=None,
                    op0=mybir.AluOpType.mult,
                )
                nc.vector.tensor_tensor(
                    out=ot[:], in0=ot[:], in1=xr[:], op=mybir.AluOpType.add
                )
                nc.sync.dma_start(out=out_d[ti], in_=ot[:])

        ets = {}
        oa_by_ic = {}
        pts = {}

        # ---- i-chunk 0: energies/exps run at full cadence while all prep
        # (v chunks, remaining w) flows through the oa psum banks; the attn@v
        # matmuls for ic 0 are deferred until the oa banks are free ----
        ets[(0, 0)] = emit_energy(0, 0)
        for gi in range(ngroups):
            if gi + 1 < ngroups:
                ets[(0, gi + 1)] = emit_energy(0, gi + 1)
            pts[gi] = emit_exp(0, gi)
            for kind, idx, tag in prep_at.get(gi, ()):
                if kind == "v":
                    emit_vprep(idx, tag)
                elif kind == "w":
                    emit_wprep(idx, tag)
                else:
                    emit_wprep(idx, tag, half=1)
        ets[(1, 0)] = emit_energy(1, 0)
        alloc_oa(0)
        for gi in range(ngroups):
            emit_attnv(0, gi, pts.pop(gi))
            # keep feeding the energy pipeline through the deferred burst
            nxt = (1, gi + 1) if gi + 1 < ngroups else (2, 0)
            ets[nxt] = emit_energy(*nxt)
        emit_blend(0)

        # ---- i-chunks 1..: flat schedule with two-group PE lookahead ACROSS
        # chunk boundaries, so the scalar engine never waits on energy ----
        flat = [(icn, gi) for icn in range(1, NICH) for gi in range(ngroups)]
        for fk, (icn, gi) in enumerate(flat):
            for ahead in (1, 2, 3):
                if fk + ahead < len(flat) and flat[fk + ahead] not in ets:
                    nicn, ngi = flat[fk + ahead]
                    ets[flat[fk + ahead]] = emit_energy(nicn, ngi)
            pt = emit_exp(icn, gi)
            if gi == 0:
                alloc_oa(icn)
            emit_attnv(icn, gi, pt)
            if gi == ngroups - 1:
                emit_blend(icn)

    nc.finalize()
    return nc


def get_nc():
    if "nc" not in _NC_CACHE:
        _NC_CACHE["nc"] = _build_nc()
    return _NC_CACHE["nc"]


def make_in_maps(x, Wq, Wk, Wv, gamma):
    x = np.asarray(x, dtype=np.float32)
    Wq = np.asarray(Wq, dtype=np.float32)
    Wk = np.asarray(Wk, dtype=np.float32)
    Wv = np.asarray(Wv, dtype=np.float32)
    gamma = np.asarray(gamma, dtype=np.float32)

    xf = x.reshape(B, N, C)
    A = Wq @ Wk.T
    aw = np.ascontiguousarray(
        np.concatenate([A.T.astype(np.float16), Wv.astype(np.float16)], axis=1)
    )  # [128, 256] = [A^T | Wv]
    gam = gamma.reshape(1, 1)

    in_maps = []
    for c in range(NCORES):
        b, ih = c // 2, c % 2
        xT = xf[b].T.astype(np.float16)  # [128, 4096]
        # rotate the j-order so this core's own i-rows are columns 0:NI
        # (softmax sums over j, so any j-order works as long as v matches)
        xTr = np.ascontiguousarray(np.roll(xT, -ih * NI, axis=1))
        sl = slice(ih * NI, (ih + 1) * NI)
        in_maps.append(
            {
                "xT": xTr,
                "xres": np.ascontiguousarray(
                    xf[b][sl].reshape(NI // 128, 128, 128)
                ),
                "aw": aw,
                "gam": gam,
            }
        )
    return in_maps


def assemble_out(results):
    outs = [np.asarray(results[c]["out"]).reshape(NI, C) for c in range(NCORES)]
    full = np.stack(
        [np.concatenate([outs[2 * b], outs[2 * b + 1]], axis=0) for b in range(B)]
    )
    return full.reshape(B, Dd, Hh, Ww, C).astype(np.float32)


def kernel(x, Wq, Wk, Wv, gamma):
    from concourse.bass_utils import run_bass_kernel_spmd

    nc = get_nc()
    in_maps = make_in_maps(x, Wq, Wk, Wv, gamma)
    res = run_bass_kernel_spmd(nc, in_maps, core_ids=list(range(NCORES)))
    return assemble_out(res.results)
